# revision 21
# baseline (speedup 1.0000x reference)
# Trainium2 Bass kernel for nn_CAGAM (mamba cross-attention module).
# Data-parallel over batch: 8 samples -> 8 NeuronCores. Self-contained.
import os
import numpy as np
import ml_dtypes

import concourse.bass as bass
import concourse.bacc as bacc
import concourse.mybir as mybir
import concourse.tile as tile
from concourse.bass_utils import run_bass_kernel_spmd

F32 = mybir.dt.float32
BF16 = mybir.dt.bfloat16
AF = mybir.ActivationFunctionType
OP = mybir.AluOpType
AX = mybir.AxisListType

B, C, Hh, Ww = 8, 256, 32, 32
DSTATE, DCONV, NS = 16, 4, 4
DI = 512
DTR = 16
NX = DTR + 2 * DSTATE   # 48
L = 1024
LS = L // NS            # 256
EPS = 1e-5
NCORES = int(os.environ.get("KERNEL_CORES", "8"))
PAD = 12
XW = L + 2 * PAD
NDT = DI // 128         # 4
NCT = C // 128          # 2

_CACHE = {}


def _v(t, offset, dims):
    return bass.AP(tensor=t.tensor, offset=t.offset + offset,
                   ap=[t.ap[0]] + dims)


def _dv(t, offset, dims):
    return bass.AP(tensor=t.tensor, offset=t.offset + offset, ap=dims)


# ------------------------------------------------------------------ build
def _build():
    nc = bacc.Bacc("TRN2", target_bir_lowering=False, debug=False,
                   num_devices=NCORES)
    ins = {}

    def di(name, shape, dt=F32):
        ins[name] = nc.dram_tensor(name, shape, dt, kind="ExternalInput")

    di("xv", [128, 2 * L], BF16); di("xt", [128, 2 * L], BF16); di("xo", [128, 2 * L], BF16)
    for bk in ("m1", "m2"):
        di(f"{bk}_inWt", [128, 2 * 2 * DI], BF16)
        di(f"{bk}_inb", [128, 8])
        di(f"{bk}_outWt", [128, 4 * C], BF16)
        di(f"{bk}_bng", [1, C]); di(f"{bk}_bnb", [1, C])
        for dd in ("f", "b", "s"):
            p = f"{bk}_{dd}"
            di(f"{p}_convw", [128, NDT * DCONV])
            di(f"{p}_convb", [128, NDT])
            di(f"{p}_xprojt", [128, NDT * NX], BF16)
            di(f"{p}_dtWt", [DTR, DI], BF16)
            di(f"{p}_dtb", [128, NDT])
            di(f"{p}_acoef", [128, NDT * DSTATE])
            di(f"{p}_dvec", [128, NDT])
    for hd, n_out in (("verb", 6), ("target", 10)):
        di(f"{hd}_W1t", [128, 2 * 2 * C])
        di(f"{hd}_b1", [128, NDT])
        di(f"{hd}_W2t", [128, NDT * n_out])
        di(f"{hd}_b2", [n_out, 1])
    di("onehot", [NX, 2 * DSTATE * 128], BF16)
    di("identb", [128, 128], BF16)
    di("identf", [128, 128], F32)
    di("ones1", [1, 128])
    di("onesc", [128, 1])
    di("onescb", [128, 1], BF16)

    outs = {
        "vout": nc.dram_tensor("vout", [L, C], F32, kind="ExternalOutput"),
        "tout": nc.dram_tensor("tout", [L, C], F32, kind="ExternalOutput"),
        "overb": nc.dram_tensor("overb", [6, 1], F32, kind="ExternalOutput"),
        "otarget": nc.dram_tensor("otarget", [10, 1], F32,
                                  kind="ExternalOutput"),
    }

    import contextlib
    with tile.TileContext(nc) as tc, contextlib.ExitStack() as ctx:
        _emit(nc, tc, ins, outs, ctx)
    nc.compile()
    return nc


def _emit(nc, tc, ins, outs, ctx):
    ep = ctx.enter_context
    persist = ep(tc.tile_pool(name="persist", bufs=1))
    blockp = ep(tc.tile_pool(name="blockp", bufs=1))
    dirpar = ep(tc.tile_pool(name="dirpar", bufs=2))
    featp = ep(tc.tile_pool(name="featp", bufs=1))
    feat2p = ep(tc.tile_pool(name="feat2p", bufs=2))
    dirp = ep(tc.tile_pool(name="dirp", bufs=1))
    pool2 = ep(tc.tile_pool(name="pool2", bufs=2))
    pool3 = ep(tc.tile_pool(name="pool3", bufs=2))
    pool16 = ep(tc.tile_pool(name="pool16", bufs=16))
    rowp = ep(tc.tile_pool(name="rowp", bufs=4))
    smallp = ep(tc.tile_pool(name="smallp", bufs=1))
    ps_y = ep(tc.tile_pool(name="ps_y", bufs=1, space="PSUM"))
    ps_g = ep(tc.tile_pool(name="ps_g", bufs=2, space="PSUM"))
    dram = ep(tc.tile_pool(name="dram", bufs=1, space="DRAM"))

    def load(name, shape, dt=F32, pool=persist, tag=None):
        t = pool.tile(shape, dt, tag=tag or f"ld_{name}", name=f"t_{name}")
        nc.gpsimd.dma_start(t, ins[name][:])
        return t

    onehot = load("onehot", [NX, 2 * DSTATE * 128], BF16)
    identb = load("identb", [128, 128], BF16)
    identf = load("identf", [128, 128], F32)
    ones1 = load("ones1", [1, 128])
    onesc = load("onesc", [128, 1])
    onescb = load("onescb", [128, 1], BF16)

    par = {}
    for bk in ("m1", "m2"):
        par[f"{bk}_bng"] = load(f"{bk}_bng", [1, C])
        par[f"{bk}_bnb"] = load(f"{bk}_bnb", [1, C])

    epsb = persist.tile([128, 1], F32, name="epsb")
    nc.vector.memset(epsb, EPS)
    row_bounce = dram.tile([1, L], F32)
    stat_bounce = dram.tile([1, 2 * C], F32)
    stat_shared = {bk: dram.tile([1, 2 * C], F32, name=f"statsh_{bk}") for bk in ("m1", "m2")}
    pool_bounce = {bk: dram.tile([1, C], F32, name=f"poolb_{bk}") for bk in ("m1", "m2")}

    def colsum(srcs, dst_row_ap, ones=None):
        pg = ps_g.tile([128, 512], F32, tag="g")
        for ct, s in enumerate(srcs):
            nc.tensor.matmul(pg[0:1, :], ones if ones is not None else onesc,
                             s, start=(ct == 0), stop=(ct == len(srcs) - 1))
        nc.scalar.copy(dst_row_ap, pg[0:1, :])

    def replicate_row(row_ap, ncols, tag, dt=F32):
        rep = pool2.tile([128, ncols], dt, tag=tag)
        for j in range(0, ncols, 512):
            w = min(512, ncols - j)
            pr = ps_g.tile([128, 512], F32, tag="g")
            nc.tensor.matmul(pr[:, 0:w], ones1, row_ap[:, j:j + w],
                             start=True, stop=True)
            nc.scalar.activation(rep[:, j:j + w], pr[:, 0:w], AF.Identity)
        return rep

    # ============================================== scan direction
    def direction(bk, dd, xbuf, zs, yg):
        p = f"{bk}_{dd}"
        convw = load(f"{p}_convw", [128, NDT * DCONV], pool=dirpar, tag="convw")
        convb = load(f"{p}_convb", [128, NDT], pool=dirpar, tag="convb")
        xprojt = load(f"{p}_xprojt", [128, NDT * NX], BF16, pool=dirpar,
                      tag="xprojt")
        dtWt = load(f"{p}_dtWt", [DTR, DI], BF16, pool=dirpar, tag="dtWt")
        dtb = load(f"{p}_dtb", [128, NDT], pool=dirpar, tag="dtb")
        acoef = load(f"{p}_acoef", [128, NDT * DSTATE], pool=dirpar,
                     tag="acoef")
        dvec = load(f"{p}_dvec", [128, NDT], pool=dirpar, tag="dvec")

        u = [dirp.tile([128, L], BF16, tag=f"u{dt}", name=f"u{dt}") for dt in range(NDT)]
        dlt = [dirp.tile([128, L], BF16, tag=f"dlt{dt}", name=f"dlt{dt}") for dt in range(NDT)]
        dltu = [dirp.tile([128, L], BF16, tag=f"dltu{dt}", name=f"dltu{dt}")
                for dt in range(NDT)]

        def in_view(dt, k):
            xb = xbuf[dt]
            if dd == "f":
                return _v(xb, 9 + k, [[1, L]])
            if dd == "b":
                return _v(xb, 1038 - k, [[-1, L]])
            return _v(xb, 4 * k, [[1, NS], [NS, LS]])

        for dt in range(NDT):
            e0 = e1 = nc.vector
            dst = u[dt][:] if dd != "s" else _v(u[dt], 0, [[LS, NS], [1, LS]])
            e0.tensor_scalar(out=dst, in0=in_view(dt, 0),
                             scalar1=convw[:, dt * DCONV:dt * DCONV + 1],
                             scalar2=None, op0=OP.mult)
            for k in range(1, DCONV):
                e = e0 if k % 2 == 0 else e1
                e.scalar_tensor_tensor(
                    dst, in_view(dt, k),
                    convw[:, dt * DCONV + k:dt * DCONV + k + 1],
                    dst, OP.mult, OP.add)
            nc.scalar.activation(u[dt], u[dt], AF.Silu,
                                 bias=convb[:, dt:dt + 1])

        xdbl = dirp.tile([NX, L], BF16, tag="xdbl")
        for j in range(2):
            sl = slice(j * 512, (j + 1) * 512)
            px = ps_g.tile([128, 512], F32, tag="g")
            for dt in range(NDT):
                nc.tensor.matmul(px[0:NX, :], xprojt[:, dt * NX:(dt + 1) * NX],
                                 u[dt][:, sl], start=(dt == 0),
                                 stop=(dt == NDT - 1))
            nc.scalar.activation(xdbl[:, sl], px[0:NX, :], AF.Identity)

        for dt in range(NDT):
            for j in range(2):
                sl = slice(j * 512, (j + 1) * 512)
                pd = ps_g.tile([128, 512], F32, tag="g")
                nc.tensor.matmul(pd, dtWt[:, dt * 128:(dt + 1) * 128],
                                 xdbl[0:DTR, sl], start=True, stop=True)
                nc.scalar.activation(pd, pd, AF.Exp, bias=dtb[:, dt:dt + 1])
                nc.scalar.activation(dlt[dt][:, sl], pd, AF.Ln, bias=1.0)
            nc.vector.tensor_tensor(out=dltu[dt], in0=dlt[dt], in1=u[dt],
                                    op=OP.mult)

        ypsum = [ps_y.tile([128, L], F32, tag=f"y{dt}", name=f"yps{dt}") for dt in range(3)]
        yacc3 = dirp.tile([128, L], F32, tag="yacc3")
        for n in range(DSTATE):
            reps = {}
            for bi, nm in ((0, "B"), (1, "C")):
                rep = pool3.tile([128, L], BF16, tag=f"rep{nm}", name=f"rep{nm}")
                oh = onehot[:, (bi * DSTATE + n) * 128:
                            (bi * DSTATE + n + 1) * 128]
                for j in range(2):
                    sl = slice(j * 512, (j + 1) * 512)
                    pr = ps_g.tile([128, 512], F32, tag="g")
                    nc.tensor.matmul(pr, oh, xdbl[:, sl],
                                     start=True, stop=True)
                    nc.scalar.activation(rep[:, sl], pr, AF.Identity)
                reps[nm] = rep
            for dt in range(NDT):
                dec = pool3.tile([128, L], BF16, tag="dec")
                nc.scalar.activation(
                    dec, dlt[dt], AF.Exp,
                    scale=acoef[:, dt * DSTATE + n:dt * DSTATE + n + 1])
                if dd == "s":
                    nc.gpsimd.memset(_v(dec, 0, [[LS, NS]]), 0.0)
                bb = pool3.tile([128, L], BF16, tag="bb")
                eb = nc.vector if (n + dt) % 2 == 0 else nc.gpsimd
                eb.tensor_tensor(out=bb, in0=dltu[dt], in1=reps["B"],
                                 op=OP.mult)
                hh = pool3.tile([128, L], BF16, tag="hh")
                nc.vector.tensor_tensor_scan(hh, dec, bb, 0.0, OP.mult, OP.add)
                mm = pool3.tile([128, L], BF16, tag="mm")
                if dd == "f":
                    mdst, msrc, csrc = mm[:], hh[:], reps["C"][:]
                elif dd == "b":
                    mdst = _v(mm, L - 1, [[-1, L]])
                    msrc, csrc = hh[:], reps["C"][:]
                else:
                    mdst = _v(mm, 0, [[1, NS], [NS, LS]])
                    msrc = _v(hh, 0, [[LS, NS], [1, LS]])
                    csrc = _v(reps["C"], 0, [[LS, NS], [1, LS]])
                em = nc.vector if (n + dt) % 2 == 0 else nc.gpsimd
                em.tensor_tensor(out=mdst, in0=msrc, in1=csrc, op=OP.mult)
                if dt < 3:
                    for j in range(2):
                        sl = slice(j * 512, (j + 1) * 512)
                        nc.tensor.matmul(ypsum[dt][:, sl], identb, mm[:, sl],
                                         start=(n == 0),
                                         stop=(n == DSTATE - 1))
                else:
                    if n == 0:
                        nc.gpsimd.tensor_copy(yacc3, mm)
                    else:
                        nc.gpsimd.tensor_tensor(out=yacc3, in0=yacc3, in1=mm,
                                                op=OP.add)
        # drain: yg += (D*u + y) * zs   (true-time order)
        for dt in range(NDT):
            if dd == "s":
                uv = _v(u[dt], 0, [[1, LS], [LS, NS]])
                base_t = ypsum[dt] if dt < 3 else yacc3
                ysrc = _v(base_t, 0, [[4, LS], [1, NS]])
                t1shape = [[4, LS], [1, NS]]
            else:
                uv = u[dt][:] if dd == "f" else _v(u[dt], L - 1, [[-1, L]])
                ysrc = ypsum[dt][:] if dt < 3 else yacc3[:]
                t1shape = None
            t1 = pool3.tile([128, L], BF16, tag="t1")
            t1v = t1[:] if t1shape is None else _v(t1, 0, t1shape)
            nc.vector.scalar_tensor_tensor(t1v, uv, dvec[:, dt:dt + 1], ysrc,
                                           OP.mult, OP.add)
            if dd == "f":
                nc.vector.tensor_tensor(out=yg[dt], in0=t1, in1=zs[dt],
                                        op=OP.mult)
            else:
                prod = pool3.tile([128, L], BF16, tag="prod")
                nc.vector.tensor_tensor(out=prod, in0=t1, in1=zs[dt],
                                        op=OP.mult)
                nc.vector.tensor_tensor(out=yg[dt], in0=yg[dt], in1=prod,
                                        op=OP.add)

    # ============================================== feat
    def feat(bk, x, inWt, inb):
        sq = featp.tile([128, 2 * L], BF16, tag="sq")
        for ct in range(NCT):
            nc.scalar.activation(sq[:, ct * L:(ct + 1) * L],
                                 x[:, ct * L:(ct + 1) * L], AF.Square)
        bn1 = rowp.tile([1, L], F32, tag="row")
        bn2 = rowp.tile([1, L], F32, tag="row")
        for j in range(2):
            jsl = slice(j * 512, (j + 1) * 512)
            colsum([x[:, ct * L + j * 512: ct * L + (j + 1) * 512]
                    for ct in range(NCT)], bn1[:, jsl], ones=onescb)
            colsum([sq[:, ct * L + j * 512: ct * L + (j + 1) * 512]
                    for ct in range(NCT)], bn2[:, jsl], ones=onescb)
        mrow = rowp.tile([1, L], F32, tag="row")
        nc.vector.tensor_scalar(out=mrow, in0=bn1, scalar1=1.0 / C,
                                scalar2=None, op0=OP.mult)
        vrow = rowp.tile([1, L], F32, tag="row")
        nc.vector.tensor_tensor(out=vrow, in0=mrow, in1=mrow, op=OP.mult)
        nc.vector.scalar_tensor_tensor(vrow, bn2, 1.0 / C, vrow, OP.mult,
                                       OP.subtract)
        nc.scalar.dma_start(row_bounce[:], vrow)
        v128 = smallp.tile([128, L // 128], F32, tag="v128")
        nc.gpsimd.dma_start(v128, _dv(row_bounce, 0,
                                      [[L // 128, 128], [1, L // 128]]))
        nc.scalar.activation(v128, v128, AF.Sqrt, bias=epsb)
        nc.vector.reciprocal(v128, v128)
        nc.scalar.dma_start(_dv(row_bounce, 0,
                                [[L // 128, 128], [1, L // 128]]), v128)
        rrow = rowp.tile([1, L], F32, tag="row")
        nc.gpsimd.dma_start(rrow, row_bounce[:])
        mr = rowp.tile([1, L], F32, tag="row")
        nc.vector.tensor_tensor(out=mr, in0=mrow, in1=rrow, op=OP.mult)
        r_rep = replicate_row(rrow, L, "r_rep", BF16)
        mr_rep = replicate_row(mr, L, "mr_rep", BF16)
        xhat = featp.tile([128, 2 * L], BF16, tag="xh")
        for ct in range(NCT):
            sl = slice(ct * L, (ct + 1) * L)
            nc.vector.tensor_tensor(out=xhat[:, sl], in0=x[:, sl], in1=r_rep,
                                    op=OP.mult)
            nc.vector.tensor_tensor(out=xhat[:, sl], in0=xhat[:, sl],
                                    in1=mr_rep, op=OP.subtract)

        if os.environ.get("KERNEL_STAGE", "all") == "xhat":
            for lt in range(8):
                st0 = pool3.tile([128, 128], BF16, tag="outst", name="outst3")
                nc.vector.tensor_copy(st0, xhat[:, lt * 128:(lt + 1) * 128])
                st1 = pool3.tile([128, 128], F32, tag="outstf", name="outst4")
                nc.vector.tensor_copy(st1, st0)
                nc.scalar.dma_start(
                    outs["vout"][lt * 128:(lt + 1) * 128, 0:128], st1)
            return None
        xbuf = [featp.tile([128, XW], BF16, tag=f"xbuf{dt}", name=f"xbuf{dt}")
                for dt in range(NDT)]
        zs = [featp.tile([128, L], BF16, tag=f"zs{dt}", name=f"zs{dt}") for dt in range(NDT)]
        for dt in range(NDT):
            nc.vector.memset(xbuf[dt][:, 0:PAD], 0.0)
            nc.vector.memset(xbuf[dt][:, PAD + L:XW], 0.0)
        for mt in range(8):
            for j in range(2):
                pg = ps_g.tile([128, 512], F32, tag="g")
                for ct in range(NCT):
                    nc.tensor.matmul(
                        pg,
                        inWt[:, ct * (2 * DI) + mt * 128:
                             ct * (2 * DI) + (mt + 1) * 128],
                        xhat[:, ct * L + j * 512: ct * L + (j + 1) * 512],
                        start=(ct == 0), stop=(ct == NCT - 1))
                if mt < NDT:
                    nc.scalar.activation(
                        xbuf[mt][:, PAD + j * 512:PAD + (j + 1) * 512], pg,
                        AF.Identity, bias=inb[:, mt:mt + 1])
                else:
                    nc.scalar.activation(
                        zs[mt - NDT][:, j * 512:(j + 1) * 512], pg, AF.Silu,
                        bias=inb[:, mt:mt + 1])

        yg = [featp.tile([128, L], BF16, tag=f"yg{dt}", name=f"yg{dt}") for dt in range(NDT)]
        stage = os.environ.get("KERNEL_STAGE", "all")
        dirs = ("f", "b", "s")
        if stage == "dirf":
            dirs = ("f",)
        elif stage == "dirs":
            dirs = ("s",)
        elif stage == "inproj":
            dirs = ()
            for dt in range(NDT):
                nc.vector.tensor_copy(yg[dt], zs[dt])
        for dd in dirs:
            direction(bk, dd, xbuf, zs, yg)

        outWt = blockp.tile([128, 4 * C], BF16, tag="outWt")
        nc.gpsimd.dma_start(outWt, ins[f"{bk}_outWt"][:])
        Fb = feat2p.tile([128, NCT * L], BF16, tag="Fb")
        for ct in range(NCT):
            for j in range(2):
                sl = slice(j * 512, (j + 1) * 512)
                pg = ps_g.tile([128, 512], F32, tag="g")
                for dt in range(NDT):
                    nc.tensor.matmul(pg,
                                     outWt[:, dt * C + ct * 128:
                                           dt * C + (ct + 1) * 128],
                                     yg[dt][:, sl],
                                     start=(dt == 0), stop=(dt == NDT - 1))
                nc.scalar.activation(
                    Fb[:, ct * L + j * 512: ct * L + (j + 1) * 512], pg,
                    AF.Identity)
        return Fb

    # ============================================== block
    def block(bk, xname, hd, oname):
        inWt = blockp.tile([128, 2 * 2 * DI], BF16, tag="inWt")
        nc.gpsimd.dma_start(inWt, ins[f"{bk}_inWt"][:])
        inb = blockp.tile([128, 8], F32, tag="inb")
        nc.gpsimd.dma_start(inb, ins[f"{bk}_inb"][:])
        x1 = blockp.tile([128, 2 * L], BF16, tag="x1")
        nc.gpsimd.dma_start(x1, ins[xname][:])
        x2 = blockp.tile([128, 2 * L], BF16, tag="x2")
        nc.gpsimd.dma_start(x2, ins["xo"][:])

        Afb = feat(bk, x1, inWt, inb)
        Bfb = feat(bk, x2, inWt, inb)

        dotT = featp.tile([128, NCT * C], BF16, tag="dotT")
        ATs = [[None] * NCT for _ in range(8)]
        BTs = [[None] * NCT for _ in range(8)]
        for lt in range(8):
            for ct in range(NCT):
                for src, mat, tg in ((Afb, ATs, "AT"), (Bfb, BTs, "BT")):
                    pt = ps_g.tile([128, 512], BF16, tag="g", name="ptb")
                    nc.tensor.transpose(pt[:, 0:128],
                                        src[:, ct * L + lt * 128:
                                            ct * L + (lt + 1) * 128], identb)
                    tt = pool16.tile([128, 128], BF16, tag=tg, name="tt")
                    nc.scalar.activation(tt, pt[:, 0:128], AF.Identity)
                    mat[lt][ct] = tt
        for ct2 in range(NCT):
            pdot = ps_g.tile([128, 512], F32, tag="g")
            for ct in range(NCT):
                for lt in range(8):
                    nc.tensor.matmul(pdot[:, ct * 128:(ct + 1) * 128],
                                     BTs[lt][ct2], ATs[lt][ct],
                                     start=(lt == 0), stop=(lt == 7))
            nc.vector.tensor_copy(dotT[:, ct2 * C:(ct2 + 1) * C],
                                  pdot[:, 0:C])
        res = featp.tile([128, NCT * L], F32, tag="res")
        for ct in range(NCT):
            for j in range(2):
                pg = ps_g.tile([128, 512], F32, tag="g")
                for ct2 in range(NCT):
                    nc.tensor.matmul(
                        pg, _v(dotT, ct2 * C + ct * 128, [[1, 128]]),
                        Bfb[:, ct2 * L + j * 512: ct2 * L + (j + 1) * 512],
                        start=(ct2 == 0), stop=(ct2 == NCT - 1))
                nc.scalar.copy(
                    res[:, ct * L + j * 512: ct * L + (j + 1) * 512], pg)

        rsq = featp.tile([128, 2 * L], F32, tag="sq")
        for ct in range(NCT):
            nc.scalar.activation(rsq[:, ct * L:(ct + 1) * L],
                                 res[:, ct * L:(ct + 1) * L], AF.Square)
        bs1 = rowp.tile([1, L], F32, tag="row")
        bs2 = rowp.tile([1, L], F32, tag="row")
        for j in range(2):
            jsl = slice(j * 512, (j + 1) * 512)
            colsum([res[:, ct * L + j * 512: ct * L + (j + 1) * 512]
                    for ct in range(NCT)], bs1[:, jsl])
            colsum([rsq[:, ct * L + j * 512: ct * L + (j + 1) * 512]
                    for ct in range(NCT)], bs2[:, jsl])
        srow = rowp.tile([1, 2 * C], F32, tag="row")
        sloc = smallp.tile([1, C], F32, tag="bnsloc")
        nc.vector.tensor_reduce(out=srow[:, 0:C],
                                in_=_v(bs1, 0, [[4, C], [1, 4]]),
                                axis=AX.X, op=OP.add)
        nc.vector.tensor_reduce(out=srow[:, C:2 * C],
                                in_=_v(bs2, 0, [[4, C], [1, 4]]),
                                axis=AX.X, op=OP.add)
        nc.vector.tensor_copy(sloc, srow[:, 0:C])
        nc.scalar.dma_start(stat_bounce[:], srow[:, 0:2 * C])
        if os.environ.get("KERNEL_NOCC"):
            nc.gpsimd.dma_start(stat_shared[bk][:], stat_bounce[:])
        else:
            nc.gpsimd.collective_compute(
                "AllReduce", OP.add, replica_groups=[list(range(NCORES))],
                ins=[stat_bounce[:]], outs=[stat_shared[bk][:]])
        glob = smallp.tile([1, 2 * C], F32, tag="bnglob")
        nc.gpsimd.dma_start(glob, stat_shared[bk][:])
        den = 1.0 / (B * L)
        gm = smallp.tile([1, C], F32, tag="bngm")
        nc.vector.tensor_scalar(out=gm, in0=glob[:, 0:C], scalar1=den,
                                scalar2=None, op0=OP.mult)
        gvar = smallp.tile([1, C], F32, tag="bngvar")
        nc.vector.tensor_tensor(out=gvar, in0=gm, in1=gm, op=OP.mult)
        nc.vector.scalar_tensor_tensor(gvar, glob[:, C:2 * C], den, gvar,
                                       OP.mult, OP.subtract)
        nc.scalar.activation(gvar, gvar, AF.Sqrt, bias=epsb[0:1, :])
        gA = smallp.tile([1, C], F32, tag="bngA")
        nc.vector.reciprocal(gA, gvar)
        nc.vector.tensor_tensor(out=gA, in0=gA, in1=par[f"{bk}_bng"],
                                op=OP.mult)
        gB = smallp.tile([1, C], F32, tag="bngB")
        nc.vector.tensor_tensor(out=gB, in0=gm, in1=gA, op=OP.mult)
        nc.vector.tensor_tensor(out=gB, in0=par[f"{bk}_bnb"], in1=gB,
                                op=OP.subtract)
        A_rep = replicate_row(gA, C, "bnArep")
        B_rep = replicate_row(gB, C, "bnBrep")
        for ct in range(NCT):
            rv = _v(res, ct * L, [[4, 256], [1, 4]])
            va = _v(A_rep, 0, [[1, 256], [0, 4]])
            vb = _v(B_rep, 0, [[1, 256], [0, 4]])
            nc.vector.tensor_tensor(out=rv, in0=rv, in1=va, op=OP.mult)
            nc.vector.tensor_tensor(out=rv, in0=rv, in1=vb, op=OP.add)
        for lt in range(8):
            for ct in range(NCT):
                pt = ps_g.tile([128, 512], F32, tag="g")
                nc.tensor.transpose(pt[:, 0:128],
                                    res[:, ct * L + lt * 128:
                                        ct * L + (lt + 1) * 128], identf)
                st = pool3.tile([128, 128], F32, tag="outst", name="outst")
                nc.scalar.copy(st, pt[:, 0:128])
                nc.scalar.dma_start(
                    outs[oname][lt * 128:(lt + 1) * 128,
                                ct * 128:(ct + 1) * 128], st)

        # head
        pool_r = smallp.tile([1, C], F32, tag="poolr")
        nc.vector.tensor_tensor(out=pool_r, in0=sloc, in1=gA, op=OP.mult)
        nc.vector.scalar_tensor_tensor(pool_r, pool_r, 1.0 / L, gB, OP.mult,
                                       OP.add)
        nc.scalar.dma_start(pool_bounce[bk][:], pool_r)
        pool_c = smallp.tile([128, NCT], F32, tag="poolc")
        for ct in range(NCT):
            nc.gpsimd.dma_start(
                pool_c[:, ct:ct + 1],
                _dv(pool_bounce[bk], ct * 128, [[1, 128], [1, 1]]))
        n_out = 6 if hd == "verb" else 10
        W1t = load(f"{hd}_W1t", [128, 2 * 2 * C], pool=dirpar, tag="W1t")
        b1 = load(f"{hd}_b1", [128, NDT], pool=dirpar, tag="b1")
        W2t = load(f"{hd}_W2t", [128, NDT * n_out], pool=dirpar, tag="W2t")
        b2 = load(f"{hd}_b2", [n_out, 1], pool=dirpar, tag="b2")
        ph = ps_g.tile([128, 512], F32, tag="g")
        for mt in range(NDT):
            for ct in range(NCT):
                nc.tensor.matmul(ph[:, mt:mt + 1],
                                 W1t[:, ct * 2 * C + mt * 128:
                                     ct * 2 * C + (mt + 1) * 128],
                                 pool_c[:, ct:ct + 1],
                                 start=(ct == 0), stop=(ct == NCT - 1))
        h1 = smallp.tile([128, NDT], F32, tag="h1")
        nc.vector.tensor_tensor(out=h1, in0=ph[:, 0:NDT], in1=b1, op=OP.add)
        hmin = smallp.tile([128, NDT], F32, tag="hmin")
        nc.vector.tensor_scalar(out=hmin, in0=h1, scalar1=0.0, scalar2=None,
                                op0=OP.min)
        nc.scalar.activation(hmin, hmin, AF.Exp)
        nc.vector.tensor_scalar(out=h1, in0=h1, scalar1=0.0, scalar2=None,
                                op0=OP.max)
        nc.vector.tensor_tensor(out=h1, in0=h1, in1=hmin, op=OP.add)
        nc.vector.tensor_scalar(out=h1, in0=h1, scalar1=1.0, scalar2=None,
                                op0=OP.subtract)
        ph2 = ps_g.tile([128, 512], F32, tag="g")
        for dt in range(NDT):
            nc.tensor.matmul(ph2[0:n_out, 0:1],
                             W2t[:, dt * n_out:(dt + 1) * n_out],
                             h1[:, dt:dt + 1],
                             start=(dt == 0), stop=(dt == NDT - 1))
        hout = smallp.tile([n_out, 1], F32, tag="hout")
        nc.vector.tensor_tensor(out=hout, in0=ph2[0:n_out, 0:1], in1=b2,
                                op=OP.add)
        nc.scalar.dma_start(outs["overb" if hd == "verb" else "otarget"][:],
                            hout)

    stage = os.environ.get("KERNEL_STAGE", "all")
    if stage == "all":
        block("m1", "xv", "verb", "vout")
        block("m2", "xt", "target", "tout")
    else:
        inWt = blockp.tile([128, 2 * 2 * DI], BF16, tag="inWt")
        nc.gpsimd.dma_start(inWt, ins["m1_inWt"][:])
        inb = blockp.tile([128, 8], F32, tag="inb")
        nc.gpsimd.dma_start(inb, ins["m1_inb"][:])
        x1 = blockp.tile([128, 2 * L], BF16, tag="x1")
        nc.gpsimd.dma_start(x1, ins["xv"][:])
        Fb = feat("m1", x1, inWt, inb)
        if Fb is not None:
            for lt in range(8):
                st = pool3.tile([128, 128], F32, tag="outst", name="outst2")
                nc.vector.tensor_copy(st, Fb[:, lt * 128:(lt + 1) * 128])
                nc.scalar.dma_start(
                    outs["vout"][lt * 128:(lt + 1) * 128, 0:128], st)
        # dummy-write remaining outputs so PJRT output set stays complete
        zt = pool3.tile([128, 128], F32, tag="outst", name="zt")
        nc.vector.memset(zt, 0.0)
        nc.scalar.dma_start(outs["tout"][0:128, 0:128], zt)
        nc.scalar.dma_start(outs["overb"][:], zt[0:6, 0:1])
        nc.scalar.dma_start(outs["otarget"][:], zt[0:10, 0:1])


# ------------------------------------------------------------------ host
def _prep_inputs(verb_feature, target_feature, tool_feature, params):
    def f32(a):
        return np.ascontiguousarray(np.asarray(a, dtype=np.float32))

    def bf16(a):
        return np.ascontiguousarray(np.asarray(a).astype(ml_dtypes.bfloat16))

    def split_rows(a):
        R, X = a.shape
        return np.ascontiguousarray(
            a.reshape(R // 128, 128, X).transpose(1, 0, 2).reshape(128, -1))

    base = {}
    for bk in ("m1", "m2"):
        p = params[bk]
        ln_g = f32(p["ln_g"]); ln_b = f32(p["ln_b"])
        inW = f32(p["in_W"])
        base[f"{bk}_inWt"] = bf16(split_rows(f32((inW * ln_g[None, :]).T)))
        base[f"{bk}_inb"] = f32((inW @ ln_b).reshape(8, 128).T)
        base[f"{bk}_outWt"] = bf16(split_rows(f32(p["out_W"]).T))
        base[f"{bk}_bng"] = f32(p["bn_g"]).reshape(1, C)
        base[f"{bk}_bnb"] = f32(p["bn_b"]).reshape(1, C)
        for dd in ("f", "b", "s"):
            d = p[dd]
            pre = f"{bk}_{dd}"
            base[f"{pre}_convw"] = split_rows(
                f32(d["conv_W"]).reshape(DI, DCONV))
            base[f"{pre}_convb"] = f32(d["conv_b"]).reshape(NDT, 128).T
            base[f"{pre}_xprojt"] = bf16(split_rows(f32(d["xproj_W"]).T))
            base[f"{pre}_dtWt"] = bf16(f32(d["dt_W"]).T)
            base[f"{pre}_dtb"] = f32(d["dt_b"]).reshape(NDT, 128).T
            base[f"{pre}_acoef"] = split_rows(-np.exp(f32(d["A_log"])))
            base[f"{pre}_dvec"] = f32(d["D"]).reshape(NDT, 128).T
    for hd, key, n_out in (("verb", "verb", 6), ("target", "target", 10)):
        hp = params[key]
        base[f"{hd}_W1t"] = split_rows(f32(hp["W1"]).T)
        base[f"{hd}_b1"] = f32(hp["b1"]).reshape(NDT, 128).T
        base[f"{hd}_W2t"] = split_rows(f32(hp["W2"]).T)
        base[f"{hd}_b2"] = f32(hp["b2"]).reshape(n_out, 1)
    oh = np.zeros((NX, 2 * DSTATE * 128), dtype=np.float32)
    for n in range(DSTATE):
        oh[DTR + n, n * 128:(n + 1) * 128] = 1.0
        oh[DTR + DSTATE + n, (DSTATE + n) * 128:(DSTATE + n + 1) * 128] = 1.0
    base["onehot"] = bf16(oh)
    base["identb"] = np.eye(128, dtype=ml_dtypes.bfloat16)
    base["identf"] = np.eye(128, dtype=np.float32)
    base["ones1"] = np.ones((1, 128), dtype=np.float32)
    base["onesc"] = np.ones((128, 1), dtype=np.float32)
    base["onescb"] = np.ones((128, 1), dtype=ml_dtypes.bfloat16)

    vf, tf, of = (np.asarray(t, dtype=np.float32).reshape(B, C, L)
                  for t in (verb_feature, target_feature, tool_feature))
    in_maps = []
    for b in range(B):
        m = dict(base)
        m["xv"] = bf16(split_rows(vf[b]))
        m["xt"] = bf16(split_rows(tf[b]))
        m["xo"] = bf16(split_rows(of[b]))
        in_maps.append(m)
    return in_maps


def kernel(verb_feature, target_feature, tool_feature, params):
    if "nc" not in _CACHE:
        _CACHE["nc"] = _build()
    nc = _CACHE["nc"]
    in_maps = _prep_inputs(verb_feature, target_feature, tool_feature, params)
    res = run_bass_kernel_spmd(nc, in_maps[:NCORES],
                               core_ids=list(range(NCORES)))
    outs = res.results
    verb = np.stack([outs[b]["overb"][:, 0] for b in range(B)])
    target = np.stack([outs[b]["otarget"][:, 0] for b in range(B)])
    v_out = np.stack([outs[b]["vout"].reshape(C, Hh, Ww) for b in range(B)])
    t_out = np.stack([outs[b]["tout"].reshape(C, Hh, Ww) for b in range(B)])
    return (verb, v_out, target, t_out)


# revision 24
# speedup vs baseline: 5.6170x; 5.6170x over previous
# Trainium2 Bass kernel for nn_CAGAM (mamba cross-attention module).
# Data-parallel over batch: 8 samples -> 8 NeuronCores. Self-contained.
import os
import numpy as np
import ml_dtypes

import concourse.bass as bass
import concourse.bacc as bacc
import concourse.mybir as mybir
import concourse.tile as tile
from concourse.bass_utils import run_bass_kernel_spmd

F32 = mybir.dt.float32
BF16 = mybir.dt.bfloat16
AF = mybir.ActivationFunctionType
OP = mybir.AluOpType
AX = mybir.AxisListType

B, C, Hh, Ww = 8, 256, 32, 32
DSTATE, DCONV, NS = 16, 4, 4
DI = 512
DTR = 16
NX = DTR + 2 * DSTATE   # 48
L = 1024
LS = L // NS            # 256
EPS = 1e-5
NCORES = int(os.environ.get("KERNEL_CORES", "8"))
PAD = 12
XW = L + 2 * PAD
NDT = DI // 128         # 4
NCT = C // 128          # 2

_CACHE = {}


def _v(t, offset, dims):
    return bass.AP(tensor=t.tensor, offset=t.offset + offset,
                   ap=[t.ap[0]] + dims)


def _dv(t, offset, dims):
    return bass.AP(tensor=t.tensor, offset=t.offset + offset, ap=dims)


# ------------------------------------------------------------------ build
def _build():
    nc = bacc.Bacc("TRN2", target_bir_lowering=False, debug=False,
                   num_devices=NCORES)
    ins = {}

    def di(name, shape, dt=F32):
        ins[name] = nc.dram_tensor(name, shape, dt, kind="ExternalInput")

    di("xv", [128, 2 * L], BF16); di("xt", [128, 2 * L], BF16); di("xo", [128, 2 * L], BF16)
    for bk in ("m1", "m2"):
        di(f"{bk}_inWt", [128, 2 * 2 * DI], BF16)
        di(f"{bk}_inb", [128, 8])
        di(f"{bk}_outWt", [128, 4 * C], BF16)
        di(f"{bk}_bng", [1, C]); di(f"{bk}_bnb", [1, C])
        for dd in ("f", "b", "s"):
            p = f"{bk}_{dd}"
            di(f"{p}_convw", [128, NDT * DCONV])
            di(f"{p}_convb", [128, NDT])
            di(f"{p}_xprojt", [128, NDT * NX], BF16)
            di(f"{p}_dtWt", [DTR, DI], BF16)
            di(f"{p}_dtb", [128, NDT])
            di(f"{p}_acoef", [128, NDT * DSTATE])
            di(f"{p}_dvec", [128, NDT])
    for hd, n_out in (("verb", 6), ("target", 10)):
        di(f"{hd}_W1t", [128, 2 * 2 * C])
        di(f"{hd}_b1", [128, NDT])
        di(f"{hd}_W2t", [128, NDT * n_out])
        di(f"{hd}_b2", [n_out, 1])
    di("onehot", [NX, 2 * DSTATE * 128], BF16)
    di("identb", [128, 128], BF16)
    di("identf", [128, 128], F32)
    di("ones1", [1, 128])
    di("onesc", [128, 1])
    di("onescb", [128, 1], BF16)

    outs = {
        "vout": nc.dram_tensor("vout", [L, C], F32, kind="ExternalOutput"),
        "tout": nc.dram_tensor("tout", [L, C], F32, kind="ExternalOutput"),
        "overb": nc.dram_tensor("overb", [6, 1], F32, kind="ExternalOutput"),
        "otarget": nc.dram_tensor("otarget", [10, 1], F32,
                                  kind="ExternalOutput"),
    }

    import contextlib
    with tile.TileContext(nc) as tc, contextlib.ExitStack() as ctx:
        _emit(nc, tc, ins, outs, ctx)
    nc.compile()
    return nc


def _emit(nc, tc, ins, outs, ctx):
    ep = ctx.enter_context
    persist = ep(tc.tile_pool(name="persist", bufs=1))
    blockp = ep(tc.tile_pool(name="blockp", bufs=1))
    dirpar = ep(tc.tile_pool(name="dirpar", bufs=2))
    featp = ep(tc.tile_pool(name="featp", bufs=1))
    feat2p = ep(tc.tile_pool(name="feat2p", bufs=2))
    dirp = ep(tc.tile_pool(name="dirp", bufs=1))
    pool2 = ep(tc.tile_pool(name="pool2", bufs=2))
    pool3 = ep(tc.tile_pool(name="pool3", bufs=2))
    pool16 = ep(tc.tile_pool(name="pool16", bufs=16))
    rowp = ep(tc.tile_pool(name="rowp", bufs=4))
    smallp = ep(tc.tile_pool(name="smallp", bufs=1))
    ps_y = ep(tc.tile_pool(name="ps_y", bufs=1, space="PSUM"))
    ps_g = ep(tc.tile_pool(name="ps_g", bufs=2, space="PSUM"))
    dram = ep(tc.tile_pool(name="dram", bufs=1, space="DRAM"))

    def load(name, shape, dt=F32, pool=persist, tag=None):
        t = pool.tile(shape, dt, tag=tag or f"ld_{name}", name=f"t_{name}")
        nc.gpsimd.dma_start(t, ins[name][:])
        return t

    onehot = load("onehot", [NX, 2 * DSTATE * 128], BF16)
    identb = load("identb", [128, 128], BF16)
    identf = load("identf", [128, 128], F32)
    ones1 = load("ones1", [1, 128])
    onesc = load("onesc", [128, 1])
    onescb = load("onescb", [128, 1], BF16)

    par = {}
    for bk in ("m1", "m2"):
        par[f"{bk}_bng"] = load(f"{bk}_bng", [1, C])
        par[f"{bk}_bnb"] = load(f"{bk}_bnb", [1, C])

    epsb = persist.tile([128, 1], F32, name="epsb")
    nc.vector.memset(epsb, EPS)
    row_bounce = dram.tile([1, L], F32)
    stat_bounce = dram.tile([1, 2 * C], F32)
    stat_shared = {bk: dram.tile([1, 2 * C], F32, name=f"statsh_{bk}") for bk in ("m1", "m2")}
    pool_bounce = {bk: dram.tile([1, C], F32, name=f"poolb_{bk}") for bk in ("m1", "m2")}

    def colsum(srcs, dst_row_ap, ones=None):
        pg = ps_g.tile([128, 512], F32, tag="g")
        for ct, s in enumerate(srcs):
            nc.tensor.matmul(pg[0:1, :], ones if ones is not None else onesc,
                             s, start=(ct == 0), stop=(ct == len(srcs) - 1))
        nc.scalar.copy(dst_row_ap, pg[0:1, :])

    def replicate_row(row_ap, ncols, tag, dt=F32):
        rep = pool2.tile([128, ncols], dt, tag=tag)
        for j in range(0, ncols, 512):
            w = min(512, ncols - j)
            pr = ps_g.tile([128, 512], F32, tag="g")
            nc.tensor.matmul(pr[:, 0:w], ones1, row_ap[:, j:j + w],
                             start=True, stop=True)
            nc.scalar.activation(rep[:, j:j + w], pr[:, 0:w], AF.Identity)
        return rep

    # ============================================== scan direction
    def direction(bk, dd, xbuf, zs, yg):
        p = f"{bk}_{dd}"
        convw = load(f"{p}_convw", [128, NDT * DCONV], pool=dirpar, tag="convw")
        convb = load(f"{p}_convb", [128, NDT], pool=dirpar, tag="convb")
        xprojt = load(f"{p}_xprojt", [128, NDT * NX], BF16, pool=dirpar,
                      tag="xprojt")
        dtWt = load(f"{p}_dtWt", [DTR, DI], BF16, pool=dirpar, tag="dtWt")
        dtb = load(f"{p}_dtb", [128, NDT], pool=dirpar, tag="dtb")
        acoef = load(f"{p}_acoef", [128, NDT * DSTATE], pool=dirpar,
                     tag="acoef")
        dvec = load(f"{p}_dvec", [128, NDT], pool=dirpar, tag="dvec")

        u = [dirp.tile([128, L], BF16, tag=f"u{dt}", name=f"u{dt}") for dt in range(NDT)]
        dlt = [dirp.tile([128, L], BF16, tag=f"dlt{dt}", name=f"dlt{dt}") for dt in range(NDT)]
        dltu = [dirp.tile([128, L], BF16, tag=f"dltu{dt}", name=f"dltu{dt}")
                for dt in range(NDT)]

        def in_view(dt, k):
            xb = xbuf[dt]
            if dd == "f":
                return _v(xb, 9 + k, [[1, L]])
            if dd == "b":
                return _v(xb, 1038 - k, [[-1, L]])
            return _v(xb, 4 * k, [[1, NS], [NS, LS]])

        for dt in range(NDT):
            e0 = e1 = nc.vector
            dst = u[dt][:] if dd != "s" else _v(u[dt], 0, [[LS, NS], [1, LS]])
            e0.tensor_scalar(out=dst, in0=in_view(dt, 0),
                             scalar1=convw[:, dt * DCONV:dt * DCONV + 1],
                             scalar2=None, op0=OP.mult)
            for k in range(1, DCONV):
                e = e0 if k % 2 == 0 else e1
                e.scalar_tensor_tensor(
                    dst, in_view(dt, k),
                    convw[:, dt * DCONV + k:dt * DCONV + k + 1],
                    dst, OP.mult, OP.add)
            nc.scalar.activation(u[dt], u[dt], AF.Silu,
                                 bias=convb[:, dt:dt + 1])

        xdbl = dirp.tile([NX, L], BF16, tag="xdbl")
        for j in range(2):
            sl = slice(j * 512, (j + 1) * 512)
            px = ps_g.tile([128, 512], F32, tag="g")
            for dt in range(NDT):
                nc.tensor.matmul(px[0:NX, :], xprojt[:, dt * NX:(dt + 1) * NX],
                                 u[dt][:, sl], start=(dt == 0),
                                 stop=(dt == NDT - 1))
            nc.scalar.activation(xdbl[:, sl], px[0:NX, :], AF.Identity)

        for dt in range(NDT):
            for j in range(2):
                sl = slice(j * 512, (j + 1) * 512)
                pd = ps_g.tile([128, 512], F32, tag="g")
                nc.tensor.matmul(pd, dtWt[:, dt * 128:(dt + 1) * 128],
                                 xdbl[0:DTR, sl], start=True, stop=True)
                nc.scalar.activation(pd, pd, AF.Exp, bias=dtb[:, dt:dt + 1])
                nc.scalar.activation(dlt[dt][:, sl], pd, AF.Ln, bias=1.0)
            nc.vector.tensor_tensor(out=dltu[dt], in0=dlt[dt], in1=u[dt],
                                    op=OP.mult)

        ypsum = [ps_y.tile([128, L], F32, tag=f"y{dt}", name=f"yps{dt}") for dt in range(3)]
        yacc3 = dirp.tile([128, L], F32, tag="yacc3")
        for n in range(DSTATE):
            reps = {}
            for bi, nm in ((0, "B"), (1, "C")):
                rep = pool3.tile([128, L], BF16, tag=f"rep{nm}", name=f"rep{nm}")
                oh = onehot[:, (bi * DSTATE + n) * 128:
                            (bi * DSTATE + n + 1) * 128]
                for j in range(2):
                    sl = slice(j * 512, (j + 1) * 512)
                    pr = ps_g.tile([128, 512], F32, tag="g")
                    nc.tensor.matmul(pr, oh, xdbl[:, sl],
                                     start=True, stop=True)
                    nc.scalar.activation(rep[:, sl], pr, AF.Identity)
                reps[nm] = rep
            for dt in range(NDT):
                dec = pool3.tile([128, L], BF16, tag="dec")
                nc.scalar.activation(
                    dec, dlt[dt], AF.Exp,
                    scale=acoef[:, dt * DSTATE + n:dt * DSTATE + n + 1])
                if dd == "s":
                    nc.gpsimd.memset(_v(dec, 0, [[LS, NS]]), 0.0)
                bb = pool3.tile([128, L], BF16, tag="bb")
                _nogp = os.environ.get("KERNEL_NOGP")
                eb = nc.vector if (_nogp or (n + dt) % 2 == 0) else nc.gpsimd
                eb.tensor_tensor(out=bb, in0=dltu[dt], in1=reps["B"],
                                 op=OP.mult)
                hh = pool3.tile([128, L], BF16, tag="hh")
                nc.vector.tensor_tensor_scan(hh, dec, bb, 0.0, OP.mult, OP.add)
                mm = pool3.tile([128, L], BF16, tag="mm")
                if dd == "f":
                    mdst, msrc, csrc = mm[:], hh[:], reps["C"][:]
                elif dd == "b":
                    mdst = _v(mm, L - 1, [[-1, L]])
                    msrc, csrc = hh[:], reps["C"][:]
                else:
                    mdst = _v(mm, 0, [[1, NS], [NS, LS]])
                    msrc = _v(hh, 0, [[LS, NS], [1, LS]])
                    csrc = _v(reps["C"], 0, [[LS, NS], [1, LS]])
                em = nc.vector if (_nogp or (n + dt) % 2 == 0) else nc.gpsimd
                em.tensor_tensor(out=mdst, in0=msrc, in1=csrc, op=OP.mult)
                if dt < 3:
                    for j in range(2):
                        sl = slice(j * 512, (j + 1) * 512)
                        nc.tensor.matmul(ypsum[dt][:, sl], identb, mm[:, sl],
                                         start=(n == 0),
                                         stop=(n == DSTATE - 1))
                else:
                    e3 = nc.vector if os.environ.get("KERNEL_NOGP") else nc.gpsimd
                    if n == 0:
                        e3.tensor_copy(yacc3, mm)
                    else:
                        e3.tensor_tensor(out=yacc3, in0=yacc3, in1=mm,
                                         op=OP.add)
        # drain: yg += (D*u + y) * zs   (true-time order)
        for dt in range(NDT):
            if dd == "s":
                uv = _v(u[dt], 0, [[1, LS], [LS, NS]])
                base_t = ypsum[dt] if dt < 3 else yacc3
                ysrc = _v(base_t, 0, [[4, LS], [1, NS]])
                t1shape = [[4, LS], [1, NS]]
            else:
                uv = u[dt][:] if dd == "f" else _v(u[dt], L - 1, [[-1, L]])
                ysrc = ypsum[dt][:] if dt < 3 else yacc3[:]
                t1shape = None
            t1 = pool3.tile([128, L], BF16, tag="t1")
            t1v = t1[:] if t1shape is None else _v(t1, 0, t1shape)
            nc.vector.scalar_tensor_tensor(t1v, uv, dvec[:, dt:dt + 1], ysrc,
                                           OP.mult, OP.add)
            if dd == "f":
                nc.vector.tensor_tensor(out=yg[dt], in0=t1, in1=zs[dt],
                                        op=OP.mult)
            else:
                prod = pool3.tile([128, L], BF16, tag="prod")
                nc.vector.tensor_tensor(out=prod, in0=t1, in1=zs[dt],
                                        op=OP.mult)
                nc.vector.tensor_tensor(out=yg[dt], in0=yg[dt], in1=prod,
                                        op=OP.add)

    # ============================================== feat
    def feat(bk, x, inWt, inb):
        sq = featp.tile([128, 2 * L], BF16, tag="sq")
        for ct in range(NCT):
            nc.scalar.activation(sq[:, ct * L:(ct + 1) * L],
                                 x[:, ct * L:(ct + 1) * L], AF.Square)
        bn1 = rowp.tile([1, L], F32, tag="row")
        bn2 = rowp.tile([1, L], F32, tag="row")
        for j in range(2):
            jsl = slice(j * 512, (j + 1) * 512)
            colsum([x[:, ct * L + j * 512: ct * L + (j + 1) * 512]
                    for ct in range(NCT)], bn1[:, jsl], ones=onescb)
            colsum([sq[:, ct * L + j * 512: ct * L + (j + 1) * 512]
                    for ct in range(NCT)], bn2[:, jsl], ones=onescb)
        mrow = rowp.tile([1, L], F32, tag="row")
        nc.vector.tensor_scalar(out=mrow, in0=bn1, scalar1=1.0 / C,
                                scalar2=None, op0=OP.mult)
        vrow = rowp.tile([1, L], F32, tag="row")
        nc.vector.tensor_tensor(out=vrow, in0=mrow, in1=mrow, op=OP.mult)
        nc.vector.scalar_tensor_tensor(vrow, bn2, 1.0 / C, vrow, OP.mult,
                                       OP.subtract)
        nc.scalar.dma_start(row_bounce[:], vrow)
        v128 = smallp.tile([128, L // 128], F32, tag="v128")
        nc.gpsimd.dma_start(v128, _dv(row_bounce, 0,
                                      [[L // 128, 128], [1, L // 128]]))
        nc.scalar.activation(v128, v128, AF.Sqrt, bias=epsb)
        nc.vector.reciprocal(v128, v128)
        nc.scalar.dma_start(_dv(row_bounce, 0,
                                [[L // 128, 128], [1, L // 128]]), v128)
        rrow = rowp.tile([1, L], F32, tag="row")
        nc.gpsimd.dma_start(rrow, row_bounce[:])
        mr = rowp.tile([1, L], F32, tag="row")
        nc.vector.tensor_tensor(out=mr, in0=mrow, in1=rrow, op=OP.mult)
        r_rep = replicate_row(rrow, L, "r_rep", BF16)
        mr_rep = replicate_row(mr, L, "mr_rep", BF16)
        xhat = featp.tile([128, 2 * L], BF16, tag="xh")
        for ct in range(NCT):
            sl = slice(ct * L, (ct + 1) * L)
            nc.vector.tensor_tensor(out=xhat[:, sl], in0=x[:, sl], in1=r_rep,
                                    op=OP.mult)
            nc.vector.tensor_tensor(out=xhat[:, sl], in0=xhat[:, sl],
                                    in1=mr_rep, op=OP.subtract)

        if os.environ.get("KERNEL_STAGE", "all") == "xhat":
            for lt in range(8):
                st0 = pool3.tile([128, 128], BF16, tag="outst", name="outst3")
                nc.vector.tensor_copy(st0, xhat[:, lt * 128:(lt + 1) * 128])
                st1 = pool3.tile([128, 128], F32, tag="outstf", name="outst4")
                nc.vector.tensor_copy(st1, st0)
                nc.scalar.dma_start(
                    outs["vout"][lt * 128:(lt + 1) * 128, 0:128], st1)
            return None
        xbuf = [featp.tile([128, XW], BF16, tag=f"xbuf{dt}", name=f"xbuf{dt}")
                for dt in range(NDT)]
        zs = [featp.tile([128, L], BF16, tag=f"zs{dt}", name=f"zs{dt}") for dt in range(NDT)]
        for dt in range(NDT):
            nc.vector.memset(xbuf[dt][:, 0:PAD], 0.0)
            nc.vector.memset(xbuf[dt][:, PAD + L:XW], 0.0)
        for mt in range(8):
            for j in range(2):
                pg = ps_g.tile([128, 512], F32, tag="g")
                for ct in range(NCT):
                    nc.tensor.matmul(
                        pg,
                        inWt[:, ct * (2 * DI) + mt * 128:
                             ct * (2 * DI) + (mt + 1) * 128],
                        xhat[:, ct * L + j * 512: ct * L + (j + 1) * 512],
                        start=(ct == 0), stop=(ct == NCT - 1))
                if mt < NDT:
                    nc.scalar.activation(
                        xbuf[mt][:, PAD + j * 512:PAD + (j + 1) * 512], pg,
                        AF.Identity, bias=inb[:, mt:mt + 1])
                else:
                    nc.scalar.activation(
                        zs[mt - NDT][:, j * 512:(j + 1) * 512], pg, AF.Silu,
                        bias=inb[:, mt:mt + 1])

        yg = [featp.tile([128, L], BF16, tag=f"yg{dt}", name=f"yg{dt}") for dt in range(NDT)]
        stage = os.environ.get("KERNEL_STAGE", "all")
        dirs = ("f", "b", "s")
        if stage == "dirf":
            dirs = ("f",)
        elif stage == "dirs":
            dirs = ("s",)
        elif stage == "inproj":
            dirs = ()
            for dt in range(NDT):
                nc.vector.tensor_copy(yg[dt], zs[dt])
        for dd in dirs:
            direction(bk, dd, xbuf, zs, yg)

        outWt = blockp.tile([128, 4 * C], BF16, tag="outWt")
        nc.gpsimd.dma_start(outWt, ins[f"{bk}_outWt"][:])
        Fb = feat2p.tile([128, NCT * L], BF16, tag="Fb")
        for ct in range(NCT):
            for j in range(2):
                sl = slice(j * 512, (j + 1) * 512)
                pg = ps_g.tile([128, 512], F32, tag="g")
                for dt in range(NDT):
                    nc.tensor.matmul(pg,
                                     outWt[:, dt * C + ct * 128:
                                           dt * C + (ct + 1) * 128],
                                     yg[dt][:, sl],
                                     start=(dt == 0), stop=(dt == NDT - 1))
                nc.scalar.activation(
                    Fb[:, ct * L + j * 512: ct * L + (j + 1) * 512], pg,
                    AF.Identity)
        return Fb

    # ============================================== block
    def block(bk, xname, hd, oname):
        inWt = blockp.tile([128, 2 * 2 * DI], BF16, tag="inWt")
        nc.gpsimd.dma_start(inWt, ins[f"{bk}_inWt"][:])
        inb = blockp.tile([128, 8], F32, tag="inb")
        nc.gpsimd.dma_start(inb, ins[f"{bk}_inb"][:])
        x1 = blockp.tile([128, 2 * L], BF16, tag="x1")
        nc.gpsimd.dma_start(x1, ins[xname][:])
        x2 = blockp.tile([128, 2 * L], BF16, tag="x2")
        nc.gpsimd.dma_start(x2, ins["xo"][:])

        Afb = feat(bk, x1, inWt, inb)
        Bfb = feat(bk, x2, inWt, inb)

        dotT = featp.tile([128, NCT * C], BF16, tag="dotT")
        ATs = [[None] * NCT for _ in range(8)]
        BTs = [[None] * NCT for _ in range(8)]
        for lt in range(8):
            for ct in range(NCT):
                for src, mat, tg in ((Afb, ATs, "AT"), (Bfb, BTs, "BT")):
                    pt = ps_g.tile([128, 512], BF16, tag="g", name="ptb")
                    nc.tensor.transpose(pt[:, 0:128],
                                        src[:, ct * L + lt * 128:
                                            ct * L + (lt + 1) * 128], identb)
                    tt = pool16.tile([128, 128], BF16, tag=tg, name="tt")
                    nc.scalar.activation(tt, pt[:, 0:128], AF.Identity)
                    mat[lt][ct] = tt
        for ct2 in range(NCT):
            pdot = ps_g.tile([128, 512], F32, tag="g")
            for ct in range(NCT):
                for lt in range(8):
                    nc.tensor.matmul(pdot[:, ct * 128:(ct + 1) * 128],
                                     BTs[lt][ct2], ATs[lt][ct],
                                     start=(lt == 0), stop=(lt == 7))
            nc.vector.tensor_copy(dotT[:, ct2 * C:(ct2 + 1) * C],
                                  pdot[:, 0:C])
        res = featp.tile([128, NCT * L], F32, tag="res")
        for ct in range(NCT):
            for j in range(2):
                pg = ps_g.tile([128, 512], F32, tag="g")
                for ct2 in range(NCT):
                    nc.tensor.matmul(
                        pg, _v(dotT, ct2 * C + ct * 128, [[1, 128]]),
                        Bfb[:, ct2 * L + j * 512: ct2 * L + (j + 1) * 512],
                        start=(ct2 == 0), stop=(ct2 == NCT - 1))
                nc.scalar.copy(
                    res[:, ct * L + j * 512: ct * L + (j + 1) * 512], pg)

        rsq = featp.tile([128, 2 * L], F32, tag="sq")
        for ct in range(NCT):
            nc.scalar.activation(rsq[:, ct * L:(ct + 1) * L],
                                 res[:, ct * L:(ct + 1) * L], AF.Square)
        bs1 = rowp.tile([1, L], F32, tag="row")
        bs2 = rowp.tile([1, L], F32, tag="row")
        for j in range(2):
            jsl = slice(j * 512, (j + 1) * 512)
            colsum([res[:, ct * L + j * 512: ct * L + (j + 1) * 512]
                    for ct in range(NCT)], bs1[:, jsl])
            colsum([rsq[:, ct * L + j * 512: ct * L + (j + 1) * 512]
                    for ct in range(NCT)], bs2[:, jsl])
        srow = rowp.tile([1, 2 * C], F32, tag="row")
        sloc = smallp.tile([1, C], F32, tag="bnsloc")
        nc.vector.tensor_reduce(out=srow[:, 0:C],
                                in_=_v(bs1, 0, [[4, C], [1, 4]]),
                                axis=AX.X, op=OP.add)
        nc.vector.tensor_reduce(out=srow[:, C:2 * C],
                                in_=_v(bs2, 0, [[4, C], [1, 4]]),
                                axis=AX.X, op=OP.add)
        nc.vector.tensor_copy(sloc, srow[:, 0:C])
        nc.scalar.dma_start(stat_bounce[:], srow[:, 0:2 * C])
        if os.environ.get("KERNEL_NOCC"):
            nc.gpsimd.dma_start(stat_shared[bk][:], stat_bounce[:])
        else:
            nc.gpsimd.collective_compute(
                "AllReduce", OP.add, replica_groups=[list(range(NCORES))],
                ins=[stat_bounce[:]], outs=[stat_shared[bk][:]])
        glob = smallp.tile([1, 2 * C], F32, tag="bnglob")
        nc.gpsimd.dma_start(glob, stat_shared[bk][:])
        den = 1.0 / (B * L)
        gm = smallp.tile([1, C], F32, tag="bngm")
        nc.vector.tensor_scalar(out=gm, in0=glob[:, 0:C], scalar1=den,
                                scalar2=None, op0=OP.mult)
        gvar = smallp.tile([1, C], F32, tag="bngvar")
        nc.vector.tensor_tensor(out=gvar, in0=gm, in1=gm, op=OP.mult)
        nc.vector.scalar_tensor_tensor(gvar, glob[:, C:2 * C], den, gvar,
                                       OP.mult, OP.subtract)
        nc.scalar.activation(gvar, gvar, AF.Sqrt, bias=epsb[0:1, :])
        gA = smallp.tile([1, C], F32, tag="bngA")
        nc.vector.reciprocal(gA, gvar)
        nc.vector.tensor_tensor(out=gA, in0=gA, in1=par[f"{bk}_bng"],
                                op=OP.mult)
        gB = smallp.tile([1, C], F32, tag="bngB")
        nc.vector.tensor_tensor(out=gB, in0=gm, in1=gA, op=OP.mult)
        nc.vector.tensor_tensor(out=gB, in0=par[f"{bk}_bnb"], in1=gB,
                                op=OP.subtract)
        A_rep = replicate_row(gA, C, "bnArep")
        B_rep = replicate_row(gB, C, "bnBrep")
        for ct in range(NCT):
            rv = _v(res, ct * L, [[4, 256], [1, 4]])
            va = _v(A_rep, 0, [[1, 256], [0, 4]])
            vb = _v(B_rep, 0, [[1, 256], [0, 4]])
            nc.vector.tensor_tensor(out=rv, in0=rv, in1=va, op=OP.mult)
            nc.vector.tensor_tensor(out=rv, in0=rv, in1=vb, op=OP.add)
        for lt in range(8):
            for ct in range(NCT):
                pt = ps_g.tile([128, 512], F32, tag="g")
                nc.tensor.transpose(pt[:, 0:128],
                                    res[:, ct * L + lt * 128:
                                        ct * L + (lt + 1) * 128], identf)
                st = pool3.tile([128, 128], F32, tag="outst", name="outst")
                nc.scalar.copy(st, pt[:, 0:128])
                nc.scalar.dma_start(
                    outs[oname][lt * 128:(lt + 1) * 128,
                                ct * 128:(ct + 1) * 128], st)

        # head
        pool_r = smallp.tile([1, C], F32, tag="poolr")
        nc.vector.tensor_tensor(out=pool_r, in0=sloc, in1=gA, op=OP.mult)
        nc.vector.scalar_tensor_tensor(pool_r, pool_r, 1.0 / L, gB, OP.mult,
                                       OP.add)
        nc.scalar.dma_start(pool_bounce[bk][:], pool_r)
        pool_c = smallp.tile([128, NCT], F32, tag="poolc")
        for ct in range(NCT):
            nc.gpsimd.dma_start(
                pool_c[:, ct:ct + 1],
                _dv(pool_bounce[bk], ct * 128, [[1, 128], [1, 1]]))
        n_out = 6 if hd == "verb" else 10
        W1t = load(f"{hd}_W1t", [128, 2 * 2 * C], pool=dirpar, tag="W1t")
        b1 = load(f"{hd}_b1", [128, NDT], pool=dirpar, tag="b1")
        W2t = load(f"{hd}_W2t", [128, NDT * n_out], pool=dirpar, tag="W2t")
        b2 = load(f"{hd}_b2", [n_out, 1], pool=dirpar, tag="b2")
        ph = ps_g.tile([128, 512], F32, tag="g")
        for mt in range(NDT):
            for ct in range(NCT):
                nc.tensor.matmul(ph[:, mt:mt + 1],
                                 W1t[:, ct * 2 * C + mt * 128:
                                     ct * 2 * C + (mt + 1) * 128],
                                 pool_c[:, ct:ct + 1],
                                 start=(ct == 0), stop=(ct == NCT - 1))
        h1 = smallp.tile([128, NDT], F32, tag="h1")
        nc.vector.tensor_tensor(out=h1, in0=ph[:, 0:NDT], in1=b1, op=OP.add)
        hmin = smallp.tile([128, NDT], F32, tag="hmin")
        nc.vector.tensor_scalar(out=hmin, in0=h1, scalar1=0.0, scalar2=None,
                                op0=OP.min)
        nc.scalar.activation(hmin, hmin, AF.Exp)
        nc.vector.tensor_scalar(out=h1, in0=h1, scalar1=0.0, scalar2=None,
                                op0=OP.max)
        nc.vector.tensor_tensor(out=h1, in0=h1, in1=hmin, op=OP.add)
        nc.vector.tensor_scalar(out=h1, in0=h1, scalar1=1.0, scalar2=None,
                                op0=OP.subtract)
        ph2 = ps_g.tile([128, 512], F32, tag="g")
        for dt in range(NDT):
            nc.tensor.matmul(ph2[0:n_out, 0:1],
                             W2t[:, dt * n_out:(dt + 1) * n_out],
                             h1[:, dt:dt + 1],
                             start=(dt == 0), stop=(dt == NDT - 1))
        hout = smallp.tile([n_out, 1], F32, tag="hout")
        nc.vector.tensor_tensor(out=hout, in0=ph2[0:n_out, 0:1], in1=b2,
                                op=OP.add)
        nc.scalar.dma_start(outs["overb" if hd == "verb" else "otarget"][:],
                            hout)

    stage = os.environ.get("KERNEL_STAGE", "all")
    if stage == "all":
        block("m1", "xv", "verb", "vout")
        block("m2", "xt", "target", "tout")
    else:
        inWt = blockp.tile([128, 2 * 2 * DI], BF16, tag="inWt")
        nc.gpsimd.dma_start(inWt, ins["m1_inWt"][:])
        inb = blockp.tile([128, 8], F32, tag="inb")
        nc.gpsimd.dma_start(inb, ins["m1_inb"][:])
        x1 = blockp.tile([128, 2 * L], BF16, tag="x1")
        nc.gpsimd.dma_start(x1, ins["xv"][:])
        Fb = feat("m1", x1, inWt, inb)
        if Fb is not None:
            for lt in range(8):
                st = pool3.tile([128, 128], F32, tag="outst", name="outst2")
                nc.vector.tensor_copy(st, Fb[:, lt * 128:(lt + 1) * 128])
                nc.scalar.dma_start(
                    outs["vout"][lt * 128:(lt + 1) * 128, 0:128], st)
        # dummy-write remaining outputs so PJRT output set stays complete
        zt = pool3.tile([128, 128], F32, tag="outst", name="zt")
        nc.vector.memset(zt, 0.0)
        nc.scalar.dma_start(outs["tout"][0:128, 0:128], zt)
        nc.scalar.dma_start(outs["overb"][:], zt[0:6, 0:1])
        nc.scalar.dma_start(outs["otarget"][:], zt[0:10, 0:1])


# ------------------------------------------------------------------ host
def _prep_inputs(verb_feature, target_feature, tool_feature, params):
    def f32(a):
        return np.ascontiguousarray(np.asarray(a, dtype=np.float32))

    def bf16(a):
        return np.ascontiguousarray(np.asarray(a).astype(ml_dtypes.bfloat16))

    def split_rows(a):
        R, X = a.shape
        return np.ascontiguousarray(
            a.reshape(R // 128, 128, X).transpose(1, 0, 2).reshape(128, -1))

    base = {}
    for bk in ("m1", "m2"):
        p = params[bk]
        ln_g = f32(p["ln_g"]); ln_b = f32(p["ln_b"])
        inW = f32(p["in_W"])
        base[f"{bk}_inWt"] = bf16(split_rows(f32((inW * ln_g[None, :]).T)))
        base[f"{bk}_inb"] = f32((inW @ ln_b).reshape(8, 128).T)
        base[f"{bk}_outWt"] = bf16(split_rows(f32(p["out_W"]).T))
        base[f"{bk}_bng"] = f32(p["bn_g"]).reshape(1, C)
        base[f"{bk}_bnb"] = f32(p["bn_b"]).reshape(1, C)
        for dd in ("f", "b", "s"):
            d = p[dd]
            pre = f"{bk}_{dd}"
            base[f"{pre}_convw"] = split_rows(
                f32(d["conv_W"]).reshape(DI, DCONV))
            base[f"{pre}_convb"] = f32(d["conv_b"]).reshape(NDT, 128).T
            base[f"{pre}_xprojt"] = bf16(split_rows(f32(d["xproj_W"]).T))
            base[f"{pre}_dtWt"] = bf16(f32(d["dt_W"]).T)
            base[f"{pre}_dtb"] = f32(d["dt_b"]).reshape(NDT, 128).T
            base[f"{pre}_acoef"] = split_rows(-np.exp(f32(d["A_log"])))
            base[f"{pre}_dvec"] = f32(d["D"]).reshape(NDT, 128).T
    for hd, key, n_out in (("verb", "verb", 6), ("target", "target", 10)):
        hp = params[key]
        base[f"{hd}_W1t"] = split_rows(f32(hp["W1"]).T)
        base[f"{hd}_b1"] = f32(hp["b1"]).reshape(NDT, 128).T
        base[f"{hd}_W2t"] = split_rows(f32(hp["W2"]).T)
        base[f"{hd}_b2"] = f32(hp["b2"]).reshape(n_out, 1)
    oh = np.zeros((NX, 2 * DSTATE * 128), dtype=np.float32)
    for n in range(DSTATE):
        oh[DTR + n, n * 128:(n + 1) * 128] = 1.0
        oh[DTR + DSTATE + n, (DSTATE + n) * 128:(DSTATE + n + 1) * 128] = 1.0
    base["onehot"] = bf16(oh)
    base["identb"] = np.eye(128, dtype=ml_dtypes.bfloat16)
    base["identf"] = np.eye(128, dtype=np.float32)
    base["ones1"] = np.ones((1, 128), dtype=np.float32)
    base["onesc"] = np.ones((128, 1), dtype=np.float32)
    base["onescb"] = np.ones((128, 1), dtype=ml_dtypes.bfloat16)

    vf, tf, of = (np.asarray(t, dtype=np.float32).reshape(B, C, L)
                  for t in (verb_feature, target_feature, tool_feature))
    in_maps = []
    for b in range(B):
        m = dict(base)
        m["xv"] = bf16(split_rows(vf[b]))
        m["xt"] = bf16(split_rows(tf[b]))
        m["xo"] = bf16(split_rows(of[b]))
        in_maps.append(m)
    return in_maps


def _make_runner(nc):
    import jax
    import concourse.mybir as mybir_
    from concourse.bass2jax import _bass_exec_p, partition_id_tensor
    from jax.sharding import Mesh, PartitionSpec
    from jax.experimental.shard_map import shard_map
    import numpy as _np

    partition_name = (nc.partition_id_tensor.name
                      if nc.partition_id_tensor else None)
    in_names, out_names, out_avals, zero_outs = [], [], [], []
    for alloc in nc.m.functions[0].allocations:
        if not isinstance(alloc, mybir_.MemoryLocationSet):
            continue
        name = alloc.memorylocations[0].name
        if alloc.kind == "ExternalInput":
            if name != partition_name:
                in_names.append(name)
        elif alloc.kind == "ExternalOutput":
            out_names.append(name)
            shape = tuple(alloc.tensor_shape)
            dtype = mybir_.dt.np(alloc.dtype)
            out_avals.append(jax.core.ShapedArray(shape, dtype))
            zero_outs.append(_np.zeros(shape, dtype))
    n_params, n_outs = len(in_names), len(out_avals)
    all_in_names = list(in_names) + list(out_names)
    if partition_name is not None:
        all_in_names.append(partition_name)
    donate = tuple(range(n_params, n_params + n_outs))

    def _body(*args):
        operands = list(args)
        if partition_name is not None:
            operands.append(partition_id_tensor())
        return tuple(_bass_exec_p.bind(
            *operands, out_avals=tuple(out_avals),
            in_names=tuple(all_in_names), out_names=tuple(out_names),
            lowering_input_output_aliases=(),
            sim_require_finite=True, sim_require_nnan=True, nc=nc))

    devices = jax.devices()[:NCORES]
    mesh = Mesh(_np.asarray(devices), ("core",))
    in_specs = (PartitionSpec("core"),) * (n_params + n_outs)
    out_specs = (PartitionSpec("core"),) * n_outs
    fn = jax.jit(shard_map(_body, mesh=mesh, in_specs=in_specs,
                           out_specs=out_specs, check_rep=False),
                 donate_argnums=donate, keep_unused=True)
    return {"fn": fn, "in_names": in_names, "out_names": out_names,
            "out_avals": out_avals, "zero_outs": zero_outs}


def _run(in_maps):
    import numpy as _np
    r = _CACHE["runner"]
    n = NCORES
    concat_in = [
        _np.concatenate([_np.asarray(in_maps[c][nm]) for c in range(n)], axis=0)
        for nm in r["in_names"]]
    concat_zeros = [_np.zeros((n * z.shape[0], *z.shape[1:]), z.dtype)
                    for z in r["zero_outs"]]
    out_arrs = r["fn"](*concat_in, *concat_zeros)
    return [
        {nm: _np.asarray(out_arrs[i]).reshape(n, *r["out_avals"][i].shape)[c]
         for i, nm in enumerate(r["out_names"])}
        for c in range(n)
    ]


def kernel(verb_feature, target_feature, tool_feature, params):
    if "nc" not in _CACHE:
        _CACHE["nc"] = _build()
        _CACHE["runner"] = _make_runner(_CACHE["nc"])
    in_maps = _prep_inputs(verb_feature, target_feature, tool_feature, params)
    outs = _run(in_maps)
    verb = np.stack([outs[b]["overb"][:, 0] for b in range(B)])
    target = np.stack([outs[b]["otarget"][:, 0] for b in range(B)])
    v_out = np.stack([outs[b]["vout"].reshape(C, Hh, Ww) for b in range(B)])
    t_out = np.stack([outs[b]["tout"].reshape(C, Hh, Ww) for b in range(B)])
    return (verb, v_out, target, t_out)


def bench(verb_feature, target_feature, tool_feature, params, iters=5):
    """Time execution with device-resident inputs. Returns seconds (min)."""
    import time
    import jax
    import numpy as _np
    if "nc" not in _CACHE:
        _CACHE["nc"] = _build()
        _CACHE["runner"] = _make_runner(_CACHE["nc"])
    r = _CACHE["runner"]
    in_maps = _prep_inputs(verb_feature, target_feature, tool_feature, params)
    n = NCORES
    concat_in = [
        _np.concatenate([_np.asarray(in_maps[c][nm]) for c in range(n)],
                        axis=0) for nm in r["in_names"]]
    dev_in = [jax.device_put(a) for a in concat_in]
    zero_sets = []
    for _ in range(iters):
        zero_sets.append([jax.device_put(
            _np.zeros((n * z.shape[0], *z.shape[1:]), z.dtype))
            for z in r["zero_outs"]])
    for zs_ in zero_sets:
        for a in zs_:
            a.block_until_ready()
    for a in dev_in:
        a.block_until_ready()
    times = []
    for it in range(iters):
        t0 = time.perf_counter()
        outs = r["fn"](*dev_in, *zero_sets[it])
        for o in outs:
            o.block_until_ready()
        times.append(time.perf_counter() - t0)
    return min(times), times


# revision 25
# speedup vs baseline: 6.7793x; 1.2069x over previous
# Trainium2 Bass kernel for nn_CAGAM (mamba cross-attention module).
# Data-parallel over batch: 8 samples -> 8 NeuronCores. Self-contained.
import os
import numpy as np
import ml_dtypes

import concourse.bass as bass
import concourse.bacc as bacc
import concourse.mybir as mybir
import concourse.tile as tile
from concourse.bass_utils import run_bass_kernel_spmd

F32 = mybir.dt.float32
BF16 = mybir.dt.bfloat16
AF = mybir.ActivationFunctionType
OP = mybir.AluOpType
AX = mybir.AxisListType

B, C, Hh, Ww = 8, 256, 32, 32
DSTATE, DCONV, NS = 16, 4, 4
DI = 512
DTR = 16
NX = DTR + 2 * DSTATE   # 48
L = 1024
LS = L // NS            # 256
EPS = 1e-5
NCORES = int(os.environ.get("KERNEL_CORES", "8"))
PAD = 12
XW = L + 2 * PAD
NDT = DI // 128         # 4
NCT = C // 128          # 2

_CACHE = {}


def _v(t, offset, dims):
    return bass.AP(tensor=t.tensor, offset=t.offset + offset,
                   ap=[t.ap[0]] + dims)


def _dv(t, offset, dims):
    return bass.AP(tensor=t.tensor, offset=t.offset + offset, ap=dims)


# ------------------------------------------------------------------ build
def _build():
    nc = bacc.Bacc("TRN2", target_bir_lowering=False, debug=False,
                   num_devices=NCORES)
    ins = {}

    def di(name, shape, dt=F32):
        ins[name] = nc.dram_tensor(name, shape, dt, kind="ExternalInput")

    di("xv", [128, 2 * L], BF16); di("xt", [128, 2 * L], BF16); di("xo", [128, 2 * L], BF16)
    for bk in ("m1", "m2"):
        di(f"{bk}_inWt", [128, 2 * 2 * DI], BF16)
        di(f"{bk}_inb", [128, 8])
        di(f"{bk}_outWt", [128, 4 * C], BF16)
        di(f"{bk}_bng", [1, C]); di(f"{bk}_bnb", [1, C])
        for dd in ("f", "b", "s"):
            p = f"{bk}_{dd}"
            di(f"{p}_convw", [128, NDT * DCONV])
            di(f"{p}_convb", [128, NDT])
            di(f"{p}_xprojt", [128, NDT * NX], BF16)
            di(f"{p}_dtWt", [DTR, DI], BF16)
            di(f"{p}_dtb", [128, NDT])
            di(f"{p}_acoef", [128, NDT * DSTATE])
            di(f"{p}_dvec", [128, NDT])
    for hd, n_out in (("verb", 6), ("target", 10)):
        di(f"{hd}_W1t", [128, 2 * 2 * C])
        di(f"{hd}_b1", [128, NDT])
        di(f"{hd}_W2t", [128, NDT * n_out])
        di(f"{hd}_b2", [n_out, 1])
    di("onehot", [NX, 2 * DSTATE * 128], BF16)
    di("identb", [128, 128], BF16)
    di("identf", [128, 128], F32)
    di("ones1", [1, 128])
    di("onesc", [128, 1])
    di("onescb", [128, 1], BF16)

    outs = {
        "vout": nc.dram_tensor("vout", [L, C], F32, kind="ExternalOutput"),
        "tout": nc.dram_tensor("tout", [L, C], F32, kind="ExternalOutput"),
        "overb": nc.dram_tensor("overb", [6, 1], F32, kind="ExternalOutput"),
        "otarget": nc.dram_tensor("otarget", [10, 1], F32,
                                  kind="ExternalOutput"),
    }

    import contextlib
    with tile.TileContext(nc) as tc, contextlib.ExitStack() as ctx:
        _emit(nc, tc, ins, outs, ctx)
    nc.compile()
    return nc


def _emit(nc, tc, ins, outs, ctx):
    ep = ctx.enter_context
    persist = ep(tc.tile_pool(name="persist", bufs=1))
    blockp = ep(tc.tile_pool(name="blockp", bufs=1))
    dirpar = ep(tc.tile_pool(name="dirpar", bufs=2))
    featp = ep(tc.tile_pool(name="featp", bufs=1))
    feat2p = ep(tc.tile_pool(name="feat2p", bufs=2))
    dirp = ep(tc.tile_pool(name="dirp", bufs=1))
    pool2 = ep(tc.tile_pool(name="pool2", bufs=2))
    pool3 = ep(tc.tile_pool(name="pool3", bufs=2))
    pool16 = ep(tc.tile_pool(name="pool16", bufs=16))
    rowp = ep(tc.tile_pool(name="rowp", bufs=4))
    smallp = ep(tc.tile_pool(name="smallp", bufs=1))
    ps_y = ep(tc.tile_pool(name="ps_y", bufs=1, space="PSUM"))
    ps_g = ep(tc.tile_pool(name="ps_g", bufs=2, space="PSUM"))
    dram = ep(tc.tile_pool(name="dram", bufs=1, space="DRAM"))

    def load(name, shape, dt=F32, pool=persist, tag=None):
        t = pool.tile(shape, dt, tag=tag or f"ld_{name}", name=f"t_{name}")
        nc.gpsimd.dma_start(t, ins[name][:])
        return t

    onehot = load("onehot", [NX, 2 * DSTATE * 128], BF16)
    identb = load("identb", [128, 128], BF16)
    identf = load("identf", [128, 128], F32)
    ones1 = load("ones1", [1, 128])
    onesc = load("onesc", [128, 1])
    onescb = load("onescb", [128, 1], BF16)

    par = {}
    for bk in ("m1", "m2"):
        par[f"{bk}_bng"] = load(f"{bk}_bng", [1, C])
        par[f"{bk}_bnb"] = load(f"{bk}_bnb", [1, C])

    epsb = persist.tile([128, 1], F32, name="epsb")
    nc.vector.memset(epsb, EPS)
    row_bounce = dram.tile([1, L], F32)
    stat_bounce = dram.tile([1, 2 * C], F32)
    stat_shared = {bk: dram.tile([1, 2 * C], F32, name=f"statsh_{bk}") for bk in ("m1", "m2")}
    pool_bounce = {bk: dram.tile([1, C], F32, name=f"poolb_{bk}") for bk in ("m1", "m2")}

    def colsum(srcs, dst_row_ap, ones=None):
        pg = ps_g.tile([128, 512], F32, tag="g")
        for ct, s in enumerate(srcs):
            nc.tensor.matmul(pg[0:1, :], ones if ones is not None else onesc,
                             s, start=(ct == 0), stop=(ct == len(srcs) - 1))
        nc.scalar.copy(dst_row_ap, pg[0:1, :])

    def replicate_row(row_ap, ncols, tag, dt=F32):
        rep = pool2.tile([128, ncols], dt, tag=tag)
        for j in range(0, ncols, 512):
            w = min(512, ncols - j)
            pr = ps_g.tile([128, 512], F32, tag="g")
            nc.tensor.matmul(pr[:, 0:w], ones1, row_ap[:, j:j + w],
                             start=True, stop=True)
            nc.scalar.activation(rep[:, j:j + w], pr[:, 0:w], AF.Identity)
        return rep

    # ============================================== scan direction
    def direction(bk, dd, xbuf, zs, yg):
        p = f"{bk}_{dd}"
        convw = load(f"{p}_convw", [128, NDT * DCONV], pool=dirpar, tag="convw")
        convb = load(f"{p}_convb", [128, NDT], pool=dirpar, tag="convb")
        xprojt = load(f"{p}_xprojt", [128, NDT * NX], BF16, pool=dirpar,
                      tag="xprojt")
        dtWt = load(f"{p}_dtWt", [DTR, DI], BF16, pool=dirpar, tag="dtWt")
        dtb = load(f"{p}_dtb", [128, NDT], pool=dirpar, tag="dtb")
        acoef = load(f"{p}_acoef", [128, NDT * DSTATE], pool=dirpar,
                     tag="acoef")
        dvec = load(f"{p}_dvec", [128, NDT], pool=dirpar, tag="dvec")

        u = [dirp.tile([128, L], BF16, tag=f"u{dt}", name=f"u{dt}") for dt in range(NDT)]
        dlt = [dirp.tile([128, L], BF16, tag=f"dlt{dt}", name=f"dlt{dt}") for dt in range(NDT)]
        dltu = [dirp.tile([128, L], BF16, tag=f"dltu{dt}", name=f"dltu{dt}")
                for dt in range(NDT)]

        def in_view(dt, k):
            xb = xbuf[dt]
            if dd == "f":
                return _v(xb, 9 + k, [[1, L]])
            if dd == "b":
                return _v(xb, 1038 - k, [[-1, L]])
            return _v(xb, 4 * k, [[1, NS], [NS, LS]])

        for dt in range(NDT):
            e0 = e1 = nc.vector
            dst = u[dt][:] if dd != "s" else _v(u[dt], 0, [[LS, NS], [1, LS]])
            e0.tensor_scalar(out=dst, in0=in_view(dt, 0),
                             scalar1=convw[:, dt * DCONV:dt * DCONV + 1],
                             scalar2=None, op0=OP.mult)
            for k in range(1, DCONV):
                e = e0 if k % 2 == 0 else e1
                e.scalar_tensor_tensor(
                    dst, in_view(dt, k),
                    convw[:, dt * DCONV + k:dt * DCONV + k + 1],
                    dst, OP.mult, OP.add)
            nc.scalar.activation(u[dt], u[dt], AF.Silu,
                                 bias=convb[:, dt:dt + 1])

        xdbl = dirp.tile([NX, L], BF16, tag="xdbl")
        for j in range(2):
            sl = slice(j * 512, (j + 1) * 512)
            px = ps_g.tile([128, 512], F32, tag="g")
            for dt in range(NDT):
                nc.tensor.matmul(px[0:NX, :], xprojt[:, dt * NX:(dt + 1) * NX],
                                 u[dt][:, sl], start=(dt == 0),
                                 stop=(dt == NDT - 1))
            nc.scalar.activation(xdbl[:, sl], px[0:NX, :], AF.Identity)

        for dt in range(NDT):
            for j in range(2):
                sl = slice(j * 512, (j + 1) * 512)
                pd = ps_g.tile([128, 512], F32, tag="g")
                nc.tensor.matmul(pd, dtWt[:, dt * 128:(dt + 1) * 128],
                                 xdbl[0:DTR, sl], start=True, stop=True)
                nc.scalar.activation(pd, pd, AF.Exp, bias=dtb[:, dt:dt + 1])
                nc.scalar.activation(dlt[dt][:, sl], pd, AF.Ln, bias=1.0)
            nc.vector.tensor_tensor(out=dltu[dt], in0=dlt[dt], in1=u[dt],
                                    op=OP.mult)

        ypsum = [ps_y.tile([128, L], F32, tag=f"y{dt}", name=f"yps{dt}") for dt in range(3)]
        yacc3 = dirp.tile([128, L], F32, tag="yacc3")
        nstates = int(os.environ.get("KERNEL_NSTATES", str(DSTATE)))
        for n in range(nstates):
            reps = {}
            for bi, nm in ((0, "B"), (1, "C")):
                rep = pool3.tile([128, L], BF16, tag=f"rep{nm}", name=f"rep{nm}")
                oh = onehot[:, (bi * DSTATE + n) * 128:
                            (bi * DSTATE + n + 1) * 128]
                for j in range(2):
                    sl = slice(j * 512, (j + 1) * 512)
                    pr = ps_g.tile([128, 512], F32, tag="g")
                    nc.tensor.matmul(pr, oh, xdbl[:, sl],
                                     start=True, stop=True)
                    nc.scalar.activation(rep[:, sl], pr, AF.Identity)
                reps[nm] = rep
            for dt in range(NDT):
                dec = pool3.tile([128, L], BF16, tag="dec")
                nc.scalar.activation(
                    dec, dlt[dt], AF.Exp,
                    scale=acoef[:, dt * DSTATE + n:dt * DSTATE + n + 1])
                if dd == "s":
                    nc.gpsimd.memset(_v(dec, 0, [[LS, NS]]), 0.0)
                bb = pool3.tile([128, L], BF16, tag="bb")
                _nogp = os.environ.get("KERNEL_NOGP")
                eb = nc.vector if (_nogp or (n + dt) % 2 == 0) else nc.gpsimd
                eb.tensor_tensor(out=bb, in0=dltu[dt], in1=reps["B"],
                                 op=OP.mult)
                hh = pool3.tile([128, L], BF16, tag="hh")
                nc.vector.tensor_tensor_scan(hh, dec, bb, 0.0, OP.mult, OP.add)
                mm = pool3.tile([128, L], BF16, tag="mm")
                if dd == "f":
                    mdst, msrc, csrc = mm[:], hh[:], reps["C"][:]
                elif dd == "b":
                    mdst = _v(mm, L - 1, [[-1, L]])
                    msrc, csrc = hh[:], reps["C"][:]
                else:
                    mdst = _v(mm, 0, [[1, NS], [NS, LS]])
                    msrc = _v(hh, 0, [[LS, NS], [1, LS]])
                    csrc = _v(reps["C"], 0, [[LS, NS], [1, LS]])
                em = nc.vector if (_nogp or (n + dt) % 2 == 0) else nc.gpsimd
                em.tensor_tensor(out=mdst, in0=msrc, in1=csrc, op=OP.mult)
                if dt < 3:
                    for j in range(2):
                        sl = slice(j * 512, (j + 1) * 512)
                        nc.tensor.matmul(ypsum[dt][:, sl], identb, mm[:, sl],
                                         start=(n == 0),
                                         stop=(n == nstates - 1))
                else:
                    e3 = nc.vector if os.environ.get("KERNEL_NOGP") else nc.gpsimd
                    if n == 0:
                        e3.tensor_copy(yacc3, mm)
                    else:
                        e3.tensor_tensor(out=yacc3, in0=yacc3, in1=mm,
                                         op=OP.add)
        # drain: yg += (D*u + y) * zs   (true-time order)
        for dt in range(NDT):
            if dd == "s":
                uv = _v(u[dt], 0, [[1, LS], [LS, NS]])
                base_t = ypsum[dt] if dt < 3 else yacc3
                ysrc = _v(base_t, 0, [[4, LS], [1, NS]])
                t1shape = [[4, LS], [1, NS]]
            else:
                uv = u[dt][:] if dd == "f" else _v(u[dt], L - 1, [[-1, L]])
                ysrc = ypsum[dt][:] if dt < 3 else yacc3[:]
                t1shape = None
            t1 = pool3.tile([128, L], BF16, tag="t1")
            t1v = t1[:] if t1shape is None else _v(t1, 0, t1shape)
            nc.vector.scalar_tensor_tensor(t1v, uv, dvec[:, dt:dt + 1], ysrc,
                                           OP.mult, OP.add)
            if dd == "f":
                nc.vector.tensor_tensor(out=yg[dt], in0=t1, in1=zs[dt],
                                        op=OP.mult)
            else:
                prod = pool3.tile([128, L], BF16, tag="prod")
                nc.vector.tensor_tensor(out=prod, in0=t1, in1=zs[dt],
                                        op=OP.mult)
                nc.vector.tensor_tensor(out=yg[dt], in0=yg[dt], in1=prod,
                                        op=OP.add)

    # ============================================== feat
    def feat(bk, x, inWt, inb):
        sq = featp.tile([128, 2 * L], BF16, tag="sq")
        for ct in range(NCT):
            nc.scalar.activation(sq[:, ct * L:(ct + 1) * L],
                                 x[:, ct * L:(ct + 1) * L], AF.Square)
        bn1 = rowp.tile([1, L], F32, tag="row")
        bn2 = rowp.tile([1, L], F32, tag="row")
        for j in range(2):
            jsl = slice(j * 512, (j + 1) * 512)
            colsum([x[:, ct * L + j * 512: ct * L + (j + 1) * 512]
                    for ct in range(NCT)], bn1[:, jsl], ones=onescb)
            colsum([sq[:, ct * L + j * 512: ct * L + (j + 1) * 512]
                    for ct in range(NCT)], bn2[:, jsl], ones=onescb)
        mrow = rowp.tile([1, L], F32, tag="row")
        nc.vector.tensor_scalar(out=mrow, in0=bn1, scalar1=1.0 / C,
                                scalar2=None, op0=OP.mult)
        vrow = rowp.tile([1, L], F32, tag="row")
        nc.vector.tensor_tensor(out=vrow, in0=mrow, in1=mrow, op=OP.mult)
        nc.vector.scalar_tensor_tensor(vrow, bn2, 1.0 / C, vrow, OP.mult,
                                       OP.subtract)
        nc.scalar.dma_start(row_bounce[:], vrow)
        v128 = smallp.tile([128, L // 128], F32, tag="v128")
        nc.gpsimd.dma_start(v128, _dv(row_bounce, 0,
                                      [[L // 128, 128], [1, L // 128]]))
        nc.scalar.activation(v128, v128, AF.Sqrt, bias=epsb)
        nc.vector.reciprocal(v128, v128)
        nc.scalar.dma_start(_dv(row_bounce, 0,
                                [[L // 128, 128], [1, L // 128]]), v128)
        rrow = rowp.tile([1, L], F32, tag="row")
        nc.gpsimd.dma_start(rrow, row_bounce[:])
        mr = rowp.tile([1, L], F32, tag="row")
        nc.vector.tensor_tensor(out=mr, in0=mrow, in1=rrow, op=OP.mult)
        r_rep = replicate_row(rrow, L, "r_rep", BF16)
        mr_rep = replicate_row(mr, L, "mr_rep", BF16)
        xhat = featp.tile([128, 2 * L], BF16, tag="xh")
        for ct in range(NCT):
            sl = slice(ct * L, (ct + 1) * L)
            nc.vector.tensor_tensor(out=xhat[:, sl], in0=x[:, sl], in1=r_rep,
                                    op=OP.mult)
            nc.vector.tensor_tensor(out=xhat[:, sl], in0=xhat[:, sl],
                                    in1=mr_rep, op=OP.subtract)

        if os.environ.get("KERNEL_STAGE", "all") == "xhat":
            for lt in range(8):
                st0 = pool3.tile([128, 128], BF16, tag="outst", name="outst3")
                nc.vector.tensor_copy(st0, xhat[:, lt * 128:(lt + 1) * 128])
                st1 = pool3.tile([128, 128], F32, tag="outstf", name="outst4")
                nc.vector.tensor_copy(st1, st0)
                nc.scalar.dma_start(
                    outs["vout"][lt * 128:(lt + 1) * 128, 0:128], st1)
            return None
        xbuf = [featp.tile([128, XW], BF16, tag=f"xbuf{dt}", name=f"xbuf{dt}")
                for dt in range(NDT)]
        zs = [featp.tile([128, L], BF16, tag=f"zs{dt}", name=f"zs{dt}") for dt in range(NDT)]
        for dt in range(NDT):
            nc.vector.memset(xbuf[dt][:, 0:PAD], 0.0)
            nc.vector.memset(xbuf[dt][:, PAD + L:XW], 0.0)
        for mt in range(8):
            for j in range(2):
                pg = ps_g.tile([128, 512], F32, tag="g")
                for ct in range(NCT):
                    nc.tensor.matmul(
                        pg,
                        inWt[:, ct * (2 * DI) + mt * 128:
                             ct * (2 * DI) + (mt + 1) * 128],
                        xhat[:, ct * L + j * 512: ct * L + (j + 1) * 512],
                        start=(ct == 0), stop=(ct == NCT - 1))
                if mt < NDT:
                    nc.scalar.activation(
                        xbuf[mt][:, PAD + j * 512:PAD + (j + 1) * 512], pg,
                        AF.Identity, bias=inb[:, mt:mt + 1])
                else:
                    nc.scalar.activation(
                        zs[mt - NDT][:, j * 512:(j + 1) * 512], pg, AF.Silu,
                        bias=inb[:, mt:mt + 1])

        yg = [featp.tile([128, L], BF16, tag=f"yg{dt}", name=f"yg{dt}") for dt in range(NDT)]
        stage = os.environ.get("KERNEL_STAGE", "all")
        dirs = ("f", "b", "s")
        if stage == "dirf":
            dirs = ("f",)
        elif stage == "dirs":
            dirs = ("s",)
        elif stage == "dirfb":
            dirs = ("f", "b")
        elif stage == "inproj":
            dirs = ()
            for dt in range(NDT):
                nc.vector.tensor_copy(yg[dt], zs[dt])
        for dd in dirs:
            direction(bk, dd, xbuf, zs, yg)

        outWt = blockp.tile([128, 4 * C], BF16, tag="outWt")
        nc.gpsimd.dma_start(outWt, ins[f"{bk}_outWt"][:])
        Fb = feat2p.tile([128, NCT * L], BF16, tag="Fb")
        for ct in range(NCT):
            for j in range(2):
                sl = slice(j * 512, (j + 1) * 512)
                pg = ps_g.tile([128, 512], F32, tag="g")
                for dt in range(NDT):
                    nc.tensor.matmul(pg,
                                     outWt[:, dt * C + ct * 128:
                                           dt * C + (ct + 1) * 128],
                                     yg[dt][:, sl],
                                     start=(dt == 0), stop=(dt == NDT - 1))
                nc.scalar.activation(
                    Fb[:, ct * L + j * 512: ct * L + (j + 1) * 512], pg,
                    AF.Identity)
        return Fb

    # ============================================== block
    def block(bk, xname, hd, oname):
        inWt = blockp.tile([128, 2 * 2 * DI], BF16, tag="inWt")
        nc.gpsimd.dma_start(inWt, ins[f"{bk}_inWt"][:])
        inb = blockp.tile([128, 8], F32, tag="inb")
        nc.gpsimd.dma_start(inb, ins[f"{bk}_inb"][:])
        x1 = blockp.tile([128, 2 * L], BF16, tag="x1")
        nc.gpsimd.dma_start(x1, ins[xname][:])
        x2 = blockp.tile([128, 2 * L], BF16, tag="x2")
        nc.gpsimd.dma_start(x2, ins["xo"][:])

        Afb = feat(bk, x1, inWt, inb)
        Bfb = feat(bk, x2, inWt, inb)

        dotT = featp.tile([128, NCT * C], BF16, tag="dotT")
        ATs = [[None] * NCT for _ in range(8)]
        BTs = [[None] * NCT for _ in range(8)]
        for lt in range(8):
            for ct in range(NCT):
                for src, mat, tg in ((Afb, ATs, "AT"), (Bfb, BTs, "BT")):
                    pt = ps_g.tile([128, 512], BF16, tag="g", name="ptb")
                    nc.tensor.transpose(pt[:, 0:128],
                                        src[:, ct * L + lt * 128:
                                            ct * L + (lt + 1) * 128], identb)
                    tt = pool16.tile([128, 128], BF16, tag=tg, name="tt")
                    nc.scalar.activation(tt, pt[:, 0:128], AF.Identity)
                    mat[lt][ct] = tt
        for ct2 in range(NCT):
            pdot = ps_g.tile([128, 512], F32, tag="g")
            for ct in range(NCT):
                for lt in range(8):
                    nc.tensor.matmul(pdot[:, ct * 128:(ct + 1) * 128],
                                     BTs[lt][ct2], ATs[lt][ct],
                                     start=(lt == 0), stop=(lt == 7))
            nc.vector.tensor_copy(dotT[:, ct2 * C:(ct2 + 1) * C],
                                  pdot[:, 0:C])
        res = featp.tile([128, NCT * L], F32, tag="res")
        for ct in range(NCT):
            for j in range(2):
                pg = ps_g.tile([128, 512], F32, tag="g")
                for ct2 in range(NCT):
                    nc.tensor.matmul(
                        pg, _v(dotT, ct2 * C + ct * 128, [[1, 128]]),
                        Bfb[:, ct2 * L + j * 512: ct2 * L + (j + 1) * 512],
                        start=(ct2 == 0), stop=(ct2 == NCT - 1))
                nc.scalar.copy(
                    res[:, ct * L + j * 512: ct * L + (j + 1) * 512], pg)

        rsq = featp.tile([128, 2 * L], F32, tag="sq")
        for ct in range(NCT):
            nc.scalar.activation(rsq[:, ct * L:(ct + 1) * L],
                                 res[:, ct * L:(ct + 1) * L], AF.Square)
        bs1 = rowp.tile([1, L], F32, tag="row")
        bs2 = rowp.tile([1, L], F32, tag="row")
        for j in range(2):
            jsl = slice(j * 512, (j + 1) * 512)
            colsum([res[:, ct * L + j * 512: ct * L + (j + 1) * 512]
                    for ct in range(NCT)], bs1[:, jsl])
            colsum([rsq[:, ct * L + j * 512: ct * L + (j + 1) * 512]
                    for ct in range(NCT)], bs2[:, jsl])
        srow = rowp.tile([1, 2 * C], F32, tag="row")
        sloc = smallp.tile([1, C], F32, tag="bnsloc")
        nc.vector.tensor_reduce(out=srow[:, 0:C],
                                in_=_v(bs1, 0, [[4, C], [1, 4]]),
                                axis=AX.X, op=OP.add)
        nc.vector.tensor_reduce(out=srow[:, C:2 * C],
                                in_=_v(bs2, 0, [[4, C], [1, 4]]),
                                axis=AX.X, op=OP.add)
        nc.vector.tensor_copy(sloc, srow[:, 0:C])
        nc.scalar.dma_start(stat_bounce[:], srow[:, 0:2 * C])
        if os.environ.get("KERNEL_NOCC"):
            nc.gpsimd.dma_start(stat_shared[bk][:], stat_bounce[:])
        else:
            nc.gpsimd.collective_compute(
                "AllReduce", OP.add, replica_groups=[list(range(NCORES))],
                ins=[stat_bounce[:]], outs=[stat_shared[bk][:]])
        glob = smallp.tile([1, 2 * C], F32, tag="bnglob")
        nc.gpsimd.dma_start(glob, stat_shared[bk][:])
        den = 1.0 / (B * L)
        gm = smallp.tile([1, C], F32, tag="bngm")
        nc.vector.tensor_scalar(out=gm, in0=glob[:, 0:C], scalar1=den,
                                scalar2=None, op0=OP.mult)
        gvar = smallp.tile([1, C], F32, tag="bngvar")
        nc.vector.tensor_tensor(out=gvar, in0=gm, in1=gm, op=OP.mult)
        nc.vector.scalar_tensor_tensor(gvar, glob[:, C:2 * C], den, gvar,
                                       OP.mult, OP.subtract)
        nc.scalar.activation(gvar, gvar, AF.Sqrt, bias=epsb[0:1, :])
        gA = smallp.tile([1, C], F32, tag="bngA")
        nc.vector.reciprocal(gA, gvar)
        nc.vector.tensor_tensor(out=gA, in0=gA, in1=par[f"{bk}_bng"],
                                op=OP.mult)
        gB = smallp.tile([1, C], F32, tag="bngB")
        nc.vector.tensor_tensor(out=gB, in0=gm, in1=gA, op=OP.mult)
        nc.vector.tensor_tensor(out=gB, in0=par[f"{bk}_bnb"], in1=gB,
                                op=OP.subtract)
        A_rep = replicate_row(gA, C, "bnArep")
        B_rep = replicate_row(gB, C, "bnBrep")
        for ct in range(NCT):
            rv = _v(res, ct * L, [[4, 256], [1, 4]])
            va = _v(A_rep, 0, [[1, 256], [0, 4]])
            vb = _v(B_rep, 0, [[1, 256], [0, 4]])
            nc.vector.tensor_tensor(out=rv, in0=rv, in1=va, op=OP.mult)
            nc.vector.tensor_tensor(out=rv, in0=rv, in1=vb, op=OP.add)
        for lt in range(8):
            for ct in range(NCT):
                pt = ps_g.tile([128, 512], F32, tag="g")
                nc.tensor.transpose(pt[:, 0:128],
                                    res[:, ct * L + lt * 128:
                                        ct * L + (lt + 1) * 128], identf)
                st = pool3.tile([128, 128], F32, tag="outst", name="outst")
                nc.scalar.copy(st, pt[:, 0:128])
                nc.scalar.dma_start(
                    outs[oname][lt * 128:(lt + 1) * 128,
                                ct * 128:(ct + 1) * 128], st)

        # head
        pool_r = smallp.tile([1, C], F32, tag="poolr")
        nc.vector.tensor_tensor(out=pool_r, in0=sloc, in1=gA, op=OP.mult)
        nc.vector.scalar_tensor_tensor(pool_r, pool_r, 1.0 / L, gB, OP.mult,
                                       OP.add)
        nc.scalar.dma_start(pool_bounce[bk][:], pool_r)
        pool_c = smallp.tile([128, NCT], F32, tag="poolc")
        for ct in range(NCT):
            nc.gpsimd.dma_start(
                pool_c[:, ct:ct + 1],
                _dv(pool_bounce[bk], ct * 128, [[1, 128], [1, 1]]))
        n_out = 6 if hd == "verb" else 10
        W1t = load(f"{hd}_W1t", [128, 2 * 2 * C], pool=dirpar, tag="W1t")
        b1 = load(f"{hd}_b1", [128, NDT], pool=dirpar, tag="b1")
        W2t = load(f"{hd}_W2t", [128, NDT * n_out], pool=dirpar, tag="W2t")
        b2 = load(f"{hd}_b2", [n_out, 1], pool=dirpar, tag="b2")
        ph = ps_g.tile([128, 512], F32, tag="g")
        for mt in range(NDT):
            for ct in range(NCT):
                nc.tensor.matmul(ph[:, mt:mt + 1],
                                 W1t[:, ct * 2 * C + mt * 128:
                                     ct * 2 * C + (mt + 1) * 128],
                                 pool_c[:, ct:ct + 1],
                                 start=(ct == 0), stop=(ct == NCT - 1))
        h1 = smallp.tile([128, NDT], F32, tag="h1")
        nc.vector.tensor_tensor(out=h1, in0=ph[:, 0:NDT], in1=b1, op=OP.add)
        hmin = smallp.tile([128, NDT], F32, tag="hmin")
        nc.vector.tensor_scalar(out=hmin, in0=h1, scalar1=0.0, scalar2=None,
                                op0=OP.min)
        nc.scalar.activation(hmin, hmin, AF.Exp)
        nc.vector.tensor_scalar(out=h1, in0=h1, scalar1=0.0, scalar2=None,
                                op0=OP.max)
        nc.vector.tensor_tensor(out=h1, in0=h1, in1=hmin, op=OP.add)
        nc.vector.tensor_scalar(out=h1, in0=h1, scalar1=1.0, scalar2=None,
                                op0=OP.subtract)
        ph2 = ps_g.tile([128, 512], F32, tag="g")
        for dt in range(NDT):
            nc.tensor.matmul(ph2[0:n_out, 0:1],
                             W2t[:, dt * n_out:(dt + 1) * n_out],
                             h1[:, dt:dt + 1],
                             start=(dt == 0), stop=(dt == NDT - 1))
        hout = smallp.tile([n_out, 1], F32, tag="hout")
        nc.vector.tensor_tensor(out=hout, in0=ph2[0:n_out, 0:1], in1=b2,
                                op=OP.add)
        nc.scalar.dma_start(outs["overb" if hd == "verb" else "otarget"][:],
                            hout)

    stage = os.environ.get("KERNEL_STAGE", "all")
    if stage == "all":
        block("m1", "xv", "verb", "vout")
        block("m2", "xt", "target", "tout")
    else:
        inWt = blockp.tile([128, 2 * 2 * DI], BF16, tag="inWt")
        nc.gpsimd.dma_start(inWt, ins["m1_inWt"][:])
        inb = blockp.tile([128, 8], F32, tag="inb")
        nc.gpsimd.dma_start(inb, ins["m1_inb"][:])
        x1 = blockp.tile([128, 2 * L], BF16, tag="x1")
        nc.gpsimd.dma_start(x1, ins["xv"][:])
        Fb = feat("m1", x1, inWt, inb)
        if Fb is not None:
            for lt in range(8):
                st = pool3.tile([128, 128], F32, tag="outst", name="outst2")
                nc.vector.tensor_copy(st, Fb[:, lt * 128:(lt + 1) * 128])
                nc.scalar.dma_start(
                    outs["vout"][lt * 128:(lt + 1) * 128, 0:128], st)
        # dummy-write remaining outputs so PJRT output set stays complete
        zt = pool3.tile([128, 128], F32, tag="outst", name="zt")
        nc.vector.memset(zt, 0.0)
        nc.scalar.dma_start(outs["tout"][0:128, 0:128], zt)
        nc.scalar.dma_start(outs["overb"][:], zt[0:6, 0:1])
        nc.scalar.dma_start(outs["otarget"][:], zt[0:10, 0:1])


# ------------------------------------------------------------------ host
def _prep_inputs(verb_feature, target_feature, tool_feature, params):
    def f32(a):
        return np.ascontiguousarray(np.asarray(a, dtype=np.float32))

    def bf16(a):
        return np.ascontiguousarray(np.asarray(a).astype(ml_dtypes.bfloat16))

    def split_rows(a):
        R, X = a.shape
        return np.ascontiguousarray(
            a.reshape(R // 128, 128, X).transpose(1, 0, 2).reshape(128, -1))

    base = {}
    for bk in ("m1", "m2"):
        p = params[bk]
        ln_g = f32(p["ln_g"]); ln_b = f32(p["ln_b"])
        inW = f32(p["in_W"])
        base[f"{bk}_inWt"] = bf16(split_rows(f32((inW * ln_g[None, :]).T)))
        base[f"{bk}_inb"] = f32((inW @ ln_b).reshape(8, 128).T)
        base[f"{bk}_outWt"] = bf16(split_rows(f32(p["out_W"]).T))
        base[f"{bk}_bng"] = f32(p["bn_g"]).reshape(1, C)
        base[f"{bk}_bnb"] = f32(p["bn_b"]).reshape(1, C)
        for dd in ("f", "b", "s"):
            d = p[dd]
            pre = f"{bk}_{dd}"
            base[f"{pre}_convw"] = split_rows(
                f32(d["conv_W"]).reshape(DI, DCONV))
            base[f"{pre}_convb"] = f32(d["conv_b"]).reshape(NDT, 128).T
            base[f"{pre}_xprojt"] = bf16(split_rows(f32(d["xproj_W"]).T))
            base[f"{pre}_dtWt"] = bf16(f32(d["dt_W"]).T)
            base[f"{pre}_dtb"] = f32(d["dt_b"]).reshape(NDT, 128).T
            base[f"{pre}_acoef"] = split_rows(-np.exp(f32(d["A_log"])))
            base[f"{pre}_dvec"] = f32(d["D"]).reshape(NDT, 128).T
    for hd, key, n_out in (("verb", "verb", 6), ("target", "target", 10)):
        hp = params[key]
        base[f"{hd}_W1t"] = split_rows(f32(hp["W1"]).T)
        base[f"{hd}_b1"] = f32(hp["b1"]).reshape(NDT, 128).T
        base[f"{hd}_W2t"] = split_rows(f32(hp["W2"]).T)
        base[f"{hd}_b2"] = f32(hp["b2"]).reshape(n_out, 1)
    oh = np.zeros((NX, 2 * DSTATE * 128), dtype=np.float32)
    for n in range(DSTATE):
        oh[DTR + n, n * 128:(n + 1) * 128] = 1.0
        oh[DTR + DSTATE + n, (DSTATE + n) * 128:(DSTATE + n + 1) * 128] = 1.0
    base["onehot"] = bf16(oh)
    base["identb"] = np.eye(128, dtype=ml_dtypes.bfloat16)
    base["identf"] = np.eye(128, dtype=np.float32)
    base["ones1"] = np.ones((1, 128), dtype=np.float32)
    base["onesc"] = np.ones((128, 1), dtype=np.float32)
    base["onescb"] = np.ones((128, 1), dtype=ml_dtypes.bfloat16)

    vf, tf, of = (np.asarray(t, dtype=np.float32).reshape(B, C, L)
                  for t in (verb_feature, target_feature, tool_feature))
    in_maps = []
    for b in range(B):
        m = dict(base)
        m["xv"] = bf16(split_rows(vf[b]))
        m["xt"] = bf16(split_rows(tf[b]))
        m["xo"] = bf16(split_rows(of[b]))
        in_maps.append(m)
    return in_maps


def _make_runner(nc):
    import jax
    import concourse.mybir as mybir_
    from concourse.bass2jax import _bass_exec_p, partition_id_tensor
    from jax.sharding import Mesh, PartitionSpec
    from jax.experimental.shard_map import shard_map
    import numpy as _np

    partition_name = (nc.partition_id_tensor.name
                      if nc.partition_id_tensor else None)
    in_names, out_names, out_avals, zero_outs = [], [], [], []
    for alloc in nc.m.functions[0].allocations:
        if not isinstance(alloc, mybir_.MemoryLocationSet):
            continue
        name = alloc.memorylocations[0].name
        if alloc.kind == "ExternalInput":
            if name != partition_name:
                in_names.append(name)
        elif alloc.kind == "ExternalOutput":
            out_names.append(name)
            shape = tuple(alloc.tensor_shape)
            dtype = mybir_.dt.np(alloc.dtype)
            out_avals.append(jax.core.ShapedArray(shape, dtype))
            zero_outs.append(_np.zeros(shape, dtype))
    n_params, n_outs = len(in_names), len(out_avals)
    all_in_names = list(in_names) + list(out_names)
    if partition_name is not None:
        all_in_names.append(partition_name)
    donate = tuple(range(n_params, n_params + n_outs))

    def _body(*args):
        operands = list(args)
        if partition_name is not None:
            operands.append(partition_id_tensor())
        return tuple(_bass_exec_p.bind(
            *operands, out_avals=tuple(out_avals),
            in_names=tuple(all_in_names), out_names=tuple(out_names),
            lowering_input_output_aliases=(),
            sim_require_finite=True, sim_require_nnan=True, nc=nc))

    devices = jax.devices()[:NCORES]
    mesh = Mesh(_np.asarray(devices), ("core",))
    in_specs = (PartitionSpec("core"),) * (n_params + n_outs)
    out_specs = (PartitionSpec("core"),) * n_outs
    fn = jax.jit(shard_map(_body, mesh=mesh, in_specs=in_specs,
                           out_specs=out_specs, check_rep=False),
                 donate_argnums=donate, keep_unused=True)
    return {"fn": fn, "in_names": in_names, "out_names": out_names,
            "out_avals": out_avals, "zero_outs": zero_outs}


def _run(in_maps):
    import numpy as _np
    r = _CACHE["runner"]
    n = NCORES
    concat_in = [
        _np.concatenate([_np.asarray(in_maps[c][nm]) for c in range(n)], axis=0)
        for nm in r["in_names"]]
    concat_zeros = [_np.zeros((n * z.shape[0], *z.shape[1:]), z.dtype)
                    for z in r["zero_outs"]]
    out_arrs = r["fn"](*concat_in, *concat_zeros)
    return [
        {nm: _np.asarray(out_arrs[i]).reshape(n, *r["out_avals"][i].shape)[c]
         for i, nm in enumerate(r["out_names"])}
        for c in range(n)
    ]


def kernel(verb_feature, target_feature, tool_feature, params):
    if "nc" not in _CACHE:
        _CACHE["nc"] = _build()
        _CACHE["runner"] = _make_runner(_CACHE["nc"])
    in_maps = _prep_inputs(verb_feature, target_feature, tool_feature, params)
    outs = _run(in_maps)
    verb = np.stack([outs[b]["overb"][:, 0] for b in range(B)])
    target = np.stack([outs[b]["otarget"][:, 0] for b in range(B)])
    v_out = np.stack([outs[b]["vout"].reshape(C, Hh, Ww) for b in range(B)])
    t_out = np.stack([outs[b]["tout"].reshape(C, Hh, Ww) for b in range(B)])
    return (verb, v_out, target, t_out)


def bench(verb_feature, target_feature, tool_feature, params, iters=5):
    """Time execution with device-resident inputs. Returns seconds (min)."""
    import time
    import jax
    import numpy as _np
    if "nc" not in _CACHE:
        _CACHE["nc"] = _build()
        _CACHE["runner"] = _make_runner(_CACHE["nc"])
    r = _CACHE["runner"]
    in_maps = _prep_inputs(verb_feature, target_feature, tool_feature, params)
    n = NCORES
    concat_in = [
        _np.concatenate([_np.asarray(in_maps[c][nm]) for c in range(n)],
                        axis=0) for nm in r["in_names"]]
    dev_in = [jax.device_put(a) for a in concat_in]
    zero_sets = []
    for _ in range(iters):
        zero_sets.append([jax.device_put(
            _np.zeros((n * z.shape[0], *z.shape[1:]), z.dtype))
            for z in r["zero_outs"]])
    for zs_ in zero_sets:
        for a in zs_:
            a.block_until_ready()
    for a in dev_in:
        a.block_until_ready()
    times = []
    for it in range(iters):
        t0 = time.perf_counter()
        outs = r["fn"](*dev_in, *zero_sets[it])
        for o in outs:
            o.block_until_ready()
        times.append(time.perf_counter() - t0)
    return min(times), times
